# revision 24
# baseline (speedup 1.0000x reference)
"""Multi-head attention (B=2, S=2048, D=1024, H=16) on 8 TRN2 NeuronCores, v3.

Sharding: data-parallel over batch (2) x tensor-parallel over head groups
(4 groups of 4 heads).  Core c = (b = c // 4, g = c % 4).

v3 design (HAM-warm dense-PE schedule):
  - All q/k/v projections in bf16; qT/kT stored PER HEAD with the 64 dh rows
    duplicated to partitions 64-127 (dup via SBUF->SBUF DMA) so the two
    512-wide q-blocks of a scores tile run CONCURRENTLY in different PE
    row-groups (tile_position auto-derived from base partitions).
  - Projections are emitted as ~1024-cycle quarter-units and spread through
    the attention j-slots so the PE never idles -> HAM stays at K=8/8
    (2.4 GHz).  Empty late slots get dummy transposes to hold the clock.
  - Act engine runs exp back-to-back ([128,1024] per (head, kt)); it is the
    steady-state bottleneck (~1.1us/instr).
  - PV in [q, dh] orientation: lhsT = pt chunk [k,128q], rhs = v_ext [k,65]
    (64 v cols + ones col -> denominators land in pv col 64).
  - normalize = DVE reciprocal + per-partition tensor_scalar_mul; PE
    transpose puts normalized attn into aT [d, q] (+v bias folded in).
  - yT written as bf16 (halves output DMA); host accumulates in fp32.
"""

import os
import sys
import types
from contextlib import ExitStack

import numpy as np

D = 1024
S = 2048
C = 256          # head cols per core (4 heads x 64)
DH = 64
NH = 4           # heads per core
QG = 1024        # q-group width
NQG = S // QG    # 2
NST = S // 128   # 16 seq tiles
NSB = QG // 128  # 8 q-subtiles per group

_CACHE = {}


def _install_ntff_shim():
    try:
        import antenv.axon_hooks  # noqa: F401
        return
    except ImportError:
        pass
    try:
        from trn_agent_boot.trn_boot import _ntff_profile_via_ctypes
        hook = _ntff_profile_via_ctypes('/opt/axon/libaxon_pjrt.so')
    except Exception:
        hook = None
    mod = types.ModuleType('antenv.axon_hooks')
    mod.get_axon_ntff_profile_hook = lambda: hook
    mod.set_axon_ntff_profile_hook = lambda h: None
    sys.modules['antenv.axon_hooks'] = mod


def build_nc():
    import concourse.bacc as bacc
    import concourse.mybir as mybir
    import concourse.tile as tile
    from concourse.bass import ts, ds

    F32 = mybir.dt.float32
    F32R = mybir.dt.float32r
    BF16 = mybir.dt.bfloat16
    ACT = mybir.ActivationFunctionType

    nc = bacc.Bacc("TRN2", target_bir_lowering=False, debug=False)
    xT = nc.dram_tensor("xT", [D, S], BF16, kind="ExternalInput")
    wq = nc.dram_tensor("wq", [D, C], BF16, kind="ExternalInput")
    wk = nc.dram_tensor("wk", [D, C], BF16, kind="ExternalInput")
    wv = nc.dram_tensor("wv", [D, C], BF16, kind="ExternalInput")
    wo = nc.dram_tensor("wo", [C, D], BF16, kind="ExternalInput")
    brow = nc.dram_tensor("brow", [1, 512], BF16, kind="ExternalInput")
    ident = nc.dram_tensor("ident", [128, 128], BF16, kind="ExternalInput")
    yT = nc.dram_tensor("yT", [D, S], BF16, kind="ExternalOutput")

    with tile.TileContext(nc) as tc, ExitStack() as ctx:
        consts = ctx.enter_context(tc.tile_pool(name="consts", bufs=1))
        sbw = ctx.enter_context(tc.tile_pool(name="weights", bufs=1))
        sbx = ctx.enter_context(tc.tile_pool(name="xsb", bufs=1))
        sbqkv = ctx.enter_context(tc.tile_pool(name="qkv", bufs=1))
        sbpt = ctx.enter_context(tc.tile_pool(name="ptp", bufs=3))
        sbat = ctx.enter_context(tc.tile_pool(name="atn", bufs=2))
        sbnrm = ctx.enter_context(tc.tile_pool(name="nrm", bufs=2))
        sby = ctx.enter_context(tc.tile_pool(name="ysb", bufs=4))
        sbtmp = ctx.enter_context(tc.tile_pool(name="tmpsb", bufs=2))
        # PSUM: sc 2x[128,1024] = 4 banks, pv 1x(2x[128,260]) = 2 banks,
        #       tr 1x[128,128] = 1 bank, yp 1x[128,512] = 1 bank -> 8 total
        scp = ctx.enter_context(tc.tile_pool(name="psc", bufs=2, space="PSUM"))
        pvp = ctx.enter_context(tc.tile_pool(name="ppv", bufs=1, space="PSUM"))
        trp = ctx.enter_context(tc.tile_pool(name="ptr", bufs=1, space="PSUM"))
        ypp = ctx.enter_context(tc.tile_pool(name="pyp", bufs=1, space="PSUM"))

        # ---- constants ----
        brow_sb = consts.tile([1, 512], BF16, tag="brow", name="brow_sb")
        nc.sync.dma_start(brow_sb[:], brow[:, :])
        ones_sb = consts.tile([1, 512], BF16, tag="ones", name="ones_sb")
        nc.gpsimd.memset(ones_sb[:], 1.0)
        id_sb = consts.tile([128, 128], BF16, tag="ident", name="id_sb")
        nc.sync.dma_start(id_sb[:], ident[:, :])

        # ---- input DMAs (ordered: wk, wv, x nb0, x nb1, wq, x nb2/3, wo) --
        wk_sb = [sbw.tile([128, C], BF16, tag=f"wk{i}", name=f"wk{i}")
                 for i in range(8)]
        wv_sb = [sbw.tile([128, C], BF16, tag=f"wv{i}", name=f"wv{i}")
                 for i in range(8)]
        wq_sb = [sbw.tile([128, C], BF16, tag=f"wq{i}", name=f"wq{i}")
                 for i in range(8)]
        xt_sb = [sbx.tile([128, S], BF16, tag=f"xt{i}", name=f"xt{i}")
                 for i in range(8)]
        for i in range(8):
            nc.sync.dma_start(wk_sb[i][:], wk[ts(i, 128), :])
        for i in range(8):
            nc.sync.dma_start(xt_sb[i][:, ts(0, 512)],
                              xT[ts(i, 128), ts(0, 512)])
        for i in range(8):
            nc.sync.dma_start(wq_sb[i][:], wq[ts(i, 128), :])
        for i in range(8):
            nc.sync.dma_start(xt_sb[i][:, ts(1, 512)],
                              xT[ts(i, 128), ts(1, 512)])
        for i in range(8):
            nc.sync.dma_start(wv_sb[i][:], wv[ts(i, 128), :])
        for i in range(8):
            nc.sync.dma_start(xt_sb[i][:, ds(1024, 1024)],
                              xT[ts(i, 128), ds(1024, 1024)])
        wo_sb = []
        for i in range(2):
            t = sbw.tile([128, D], BF16, tag=f"wo{i}", name=f"wo{i}")
            nc.sync.dma_start(t[:], wo[ts(i, 128), :])
            wo_sb.append(t)

        # ---- persistent activations ----
        # per-head q/k, dh rows duplicated into partitions 64-127
        qTd_sb = [sbqkv.tile([128, S], BF16, tag=f"qTd{h}", name=f"qTd{h}")
                  for h in range(NH)]
        kTd_sb = [sbqkv.tile([128, S], BF16, tag=f"kTd{h}", name=f"kTd{h}")
                  for h in range(NH)]
        v_sb = [sbqkv.tile([128, NH * 65], BF16, tag=f"v{i}", name=f"v{i}")
                for i in range(NST)]
        aT_sb = [sbqkv.tile([128, S], BF16, tag=f"aT{i}", name=f"aT{i}")
                 for i in range(2)]

        # ones columns of v_ext (col 65h+64 = 1.0)
        for st in range(NST):
            v3 = v_sb[st][:].rearrange("p (h e) -> p h e", e=65)
            nc.gpsimd.memset(v3[:, :, 64:65], 1.0)

        # ---- projection quarter-units (~1024 PE cycles each) ----
        open_pj = {}

        def proj_unit(which, mt, nb, u, pool=None, fast_dup=False):
            """2 of the 8 k-tile matmuls of one [128,512] q/k proj stripe;
            u==3 adds the bias via a K=1 ones-row matmul, casts to bf16
            once on DVE, then sprays the per-head row-dup copies via DMA."""
            w_sb, dsts, bcol = {
                "q": (wq_sb, qTd_sb, 0), "k": (wk_sb, kTd_sb, 256)}[which]
            key = (which, mt, nb)
            if u == 0:
                p = pool if pool is not None else ypp
                open_pj[key] = p.tile([128, 512], F32,
                                      tag="sc" if p is scp else "yp",
                                      name=f"pj_{which}{mt}{nb}")
            pj = open_pj[key]
            for kt in range(2 * u, 2 * u + 2):
                nc.tensor.matmul(
                    pj[:],
                    lhsT=w_sb[kt][:, ts(mt, 128)],
                    rhs=xt_sb[kt][:, ts(nb, 512)],
                    start=(kt == 0), stop=False,
                )
            if u == 3:
                nc.tensor.matmul(
                    pj[:],
                    lhsT=brow_sb[0:1, ds(bcol + 128 * mt, 128)],
                    rhs=ones_sb[0:1, :],
                    start=False, stop=True,
                )
                tmp = sbtmp.tile([128, 512], BF16, tag="pt16",
                                 name=f"pt16_{which}{mt}{nb}")
                nc.vector.tensor_copy(tmp[:], pj[:])
                eng = nc.scalar if fast_dup else nc.sync
                for hh in range(2):
                    dst = dsts[2 * mt + hh]
                    for rep in range(2):
                        eng.dma_start(
                            dst[64 * rep:64 * rep + 64, ts(nb, 512)],
                            tmp[64 * hh:64 * hh + 64, :])
                del open_pj[key]

        open_vp = {}

        def proj_v_half(st, half, pool=None):
            """half a seq-tile of v projection (4 of 8 k-tiles)."""
            if half == 0:
                p = pool if pool is not None else ypp
                open_vp[st] = p.tile([128, C], F32,
                                     tag="sc" if p is scp else "yp",
                                     name=f"vp{st}")
            vp = open_vp[st]
            for kt in range(4 * half, 4 * half + 4):
                nc.tensor.matmul(
                    vp[:],
                    lhsT=xt_sb[kt][:, ts(st, 128)],
                    rhs=wv_sb[kt][:],
                    start=(kt == 0), stop=(kt == 7),
                )
            if half == 1:
                v3 = v_sb[st][:].rearrange("p (h e) -> p h e", e=65)
                nc.vector.tensor_copy(
                    v3[:, :, 0:64],
                    vp[:].rearrange("p (h e) -> p h e", e=64))
                del open_vp[st]

        def transpose_at(pair, qg, qt, at_tile):
            """attn [128q,128d] -> aT[pair][...] via PE transpose + copy.
            v-bias is folded into the host-side output bias (softmax rows
            sum to 1, so attn@(v+bv) = attn@v + bv)."""
            tr = trp.tile([128, 128], BF16, tag="tr", name="tr")
            nc.tensor.transpose(tr[:], at_tile[:], id_sb[:])
            nc.vector.tensor_copy(
                aT_sb[pair][:, ds(qg * QG + qt * 128, 128)], tr[:])

        def transpose_dummy():
            """PE filler to keep the HAM clock-gate open in empty slots."""
            tr = trp.tile([128, 128], BF16, tag="tr", name="trd")
            nc.tensor.transpose(tr[:], id_sb[:], id_sb[:])

        open_yp = {}

        def out_proj_half(nb, mt, p, tail=False):
            """one of the two accumulation matmuls of an out-proj stripe."""
            key = (nb, mt)
            if p == 0:
                pool = scp if tail else ypp
                open_yp[key] = pool.tile([128, 512], F32,
                                         tag="sc" if tail else "yp",
                                         name=f"yp{nb}{mt}")
            yp = open_yp[key]
            nc.tensor.matmul(
                yp[:],
                lhsT=wo_sb[p][:, ts(mt, 128)],
                rhs=aT_sb[p][:, ts(nb, 512)],
                start=(p == 0), stop=(p == 1),
            )
            if p == 1:
                yt = sby.tile([128, 512], BF16, tag="yt", name="yt")
                nc.vector.tensor_copy(yt[:], yp[:])
                eng = nc.scalar if tail else nc.sync
                eng.dma_start(yT[ts(mt, 128), ts(nb, 512)], yt[:])
                del open_yp[key]

        # ---- interleave schedule ----
        slots = {(qg, h): {} for qg in range(NQG) for h in range(NH)}

        def add_slot(qg, h, kt, fn):
            slots[(qg, h)].setdefault(kt, []).append(fn)

        # per-(qg,pair) attn tiles, filled by norm, consumed by transpose
        attn_tiles = {}

        def norm_pair_writes(qg, h, pva, pvb):
            pair = h // 2
            if (qg, pair) not in attn_tiles:
                attn_tiles[(qg, pair)] = [
                    sbat.tile([128, 128], BF16, tag=f"at{qt}", name=f"at{qt}")
                    for qt in range(NSB)]
            tiles = attn_tiles[(qg, pair)]
            col = 64 * (h % 2)
            pa3 = pva[:].rearrange("p (s e) -> p s e", e=65)
            pb3 = pvb[:].rearrange("p (s e) -> p s e", e=65)
            recip = sbnrm.tile([128, 8], F32, tag="rc", name="rc")
            nc.vector.reciprocal(recip[:, 0:4], pa3[:, :, 64])
            nc.vector.reciprocal(recip[:, 4:8], pb3[:, :, 64])
            for qs in range(NSB):
                src3 = pa3 if qs < 4 else pb3
                nc.vector.tensor_scalar_mul(
                    tiles[qs][:, col:col + 64],
                    src3[:, qs % 4, 0:64],
                    recip[:, qs:qs + 1])

        def attention_all():
            """single software pipeline over all (qg, h, kt): iteration t
            emits exp(t-1) FIRST (the act engine's coalesced PE-semaphore
            threshold then only covers work finished a full period ago),
            then scores(t), slot fillers, pv(t-2).  Flattening across head
            boundaries removes per-head pipeline drain/refill bubbles."""
            T = NQG * NH * NST
            state, pts, scs = {}, {}, {}

            def hq(t):
                head = t // NST
                return head // NH, head % NH, t % NST

            for t in range(T + 2):
                if 1 <= t <= T:
                    pt = sbpt.tile([128, QG], BF16, tag="pt", name="pt")
                    pts[t - 1] = pt
                    nc.scalar.activation(pt[:], scs.pop(t - 1)[:], ACT.Exp)
                if t < T:
                    qg, h, j = hq(t)
                    ktd, qtd = kTd_sb[h], qTd_sb[h]
                    sc = scp.tile([128, QG], F32, tag="sc", name="sc_at")
                    scs[t] = sc
                    # two q-blocks in different PE row groups -> concurrent
                    for qb in range(2):
                        rg = 64 * qb
                        nc.tensor.matmul(
                            sc[:, ts(qb, 512)],
                            lhsT=ktd[rg:rg + 64, ts(j, 128)],
                            rhs=qtd[rg:rg + 64,
                                    ds(qg * QG + qb * 512, 512)],
                            start=True, stop=True,
                        )
                    for fn in slots[(qg, h)].get(j, ()):
                        fn()
                t2 = t - 2
                if t2 >= 0:
                    qg2, h2, j2 = hq(t2)
                    if j2 == 0:
                        state[(qg2, h2)] = (
                            pvp.tile([128, 4 * 65], F32, tag="pva",
                                     name="pva"),
                            pvp.tile([128, 4 * 65], F32, tag="pvb",
                                     name="pvb"))
                    pva, pvb = state[(qg2, h2)]
                    ptt = pts.pop(t2)
                    for qs in range(NSB):
                        pvt = pva if qs < 4 else pvb
                        nc.tensor.matmul(
                            pvt[:, ds((qs % 4) * 65, 65)],
                            lhsT=ptt[:, ts(qs, 128)],
                            rhs=v_sb[j2][:, ds(65 * h2, 65)],
                            start=(j2 == 0 and qs % 4 == 0),
                            stop=(j2 == NST - 1 and qs % 4 == 3),
                        )
                    if j2 == NST - 1:
                        norm_pair_writes(qg2, h2, pva, pvb)
                        del state[(qg2, h2)]

        # ---- lead-in: only what scores(0,0) j0 needs ----
        # k mt0 nb0 (kT head0 cols 0-512) + q mt0 nb0+nb1 (qT head0, qg0).
        for u in range(4):
            proj_unit("k", 0, 0, u, pool=scp, fast_dup=True)
        for u in range(4):
            proj_unit("q", 0, 0, u, pool=ypp, fast_dup=True)
        for u in range(4):
            proj_unit("q", 0, 1, u, pool=scp, fast_dup=True)

        # ---- slot fillers ----
        # unit helpers for slot lambdas
        def k_unit(mt, nb, u):
            return lambda: proj_unit("k", mt, nb, u)

        def q_unit(mt, nb, u):
            return lambda: proj_unit("q", mt, nb, u)

        def v_full(st):
            return [lambda st=st: proj_v_half(st, 0),
                    lambda st=st: proj_v_half(st, 1)]

        # (0,0): k mt0 catch-up (nb1 by j4, nb2 by j8, nb3 by j12),
        # v st0-15 just in time, q mt1 nb0/nb1.  One psum group per slot
        # boundary (groups never interleave).
        plan00 = {
            0: [k_unit(0, 1, u) for u in range(4)],
            1: v_full(0) + v_full(1),
            2: [k_unit(0, 2, u) for u in range(4)],
            3: v_full(2) + v_full(3),
            4: v_full(4) + v_full(5),
            5: [k_unit(0, 3, u) for u in range(4)],
            6: v_full(6) + v_full(7),
            7: v_full(8) + v_full(9),
            8: v_full(10),
            9: v_full(11),
            10: v_full(12),
            11: v_full(13),
            12: v_full(14),
            13: v_full(15),
            14: [transpose_dummy],
            15: [transpose_dummy],
        }
        for j, fns in plan00.items():
            for fn in fns:
                add_slot(0, 0, j, fn)
        # (0,1): k mt1 all four nb + q mt1 nb0/nb1 (needed by (0,2))
        for j in range(16):
            nb, u = j // 4, j % 4
            add_slot(0, 1, j, k_unit(1, nb, u))
        for j in range(8):
            nb, u = j // 4, j % 4
            add_slot(0, 1, 8 + j, q_unit(1, nb, u))
        # NOTE: norm of head (qg,h) is emitted at global iteration
        # 16*head+17 = next head's j1, so transpose slots start at j>=2.
        # (0,2): transposes pair0 qg0 on j2-9; q mt0 nb2/nb3 on j10-15+
        for qt in range(NSB):
            add_slot(0, 2, 4 + qt, (lambda qt=qt:
                     transpose_at(0, 0, qt, attn_tiles[(0, 0)][qt])))
        for j in range(6):
            nb, u = 2 + j // 4, j % 4
            add_slot(0, 2, 10 + j, q_unit(0, nb, u))
        # (0,3): q mt0 nb3 tail, q mt1 nb2/nb3, dummies
        add_slot(0, 3, 0, q_unit(0, 3, 2))
        add_slot(0, 3, 1, q_unit(0, 3, 3))
        for j in range(8):
            nb, u = 2 + j // 4, j % 4
            add_slot(0, 3, 2 + j, q_unit(1, nb, u))
        for j in range(10, 16):
            add_slot(0, 3, j, transpose_dummy)
        # (1,0): transposes pair1 qg0 on j2-9; out-proj nb0 mt0-2 j10-15
        for qt in range(NSB):
            add_slot(1, 0, 4 + qt, (lambda qt=qt:
                     transpose_at(1, 0, qt, attn_tiles[(0, 1)][qt])))
        for j in range(6):
            mt, p = j // 2, j % 2
            add_slot(1, 0, 10 + j, (lambda mt=mt, p=p:
                                    out_proj_half(0, mt, p)))
        # (1,1): out-proj nb0 mt3-7, nb1 mt0-2
        for j in range(10):
            mt, p = 3 + j // 2, j % 2
            add_slot(1, 1, j, (lambda mt=mt, p=p:
                               out_proj_half(0, mt, p)))
        for j in range(6):
            mt, p = j // 2, j % 2
            add_slot(1, 1, 10 + j, (lambda mt=mt, p=p:
                                    out_proj_half(1, mt, p)))
        # (1,2): out-proj nb1 mt3; transposes pair0 qg1 j2-9; nb1 mt4-7
        add_slot(1, 2, 0, lambda: out_proj_half(1, 3, 0))
        add_slot(1, 2, 1, lambda: out_proj_half(1, 3, 1))
        for qt in range(NSB):
            add_slot(1, 2, 4 + qt, (lambda qt=qt:
                     transpose_at(0, 1, qt, attn_tiles[(1, 0)][qt])))
        for j in range(6):
            mt, p = 4 + j // 2, j % 2
            add_slot(1, 2, 10 + j, (lambda mt=mt, p=p:
                                    out_proj_half(1, mt, p)))
        # (1,3): out-proj nb1 mt7; dummies keep the clock warm
        add_slot(1, 3, 0, lambda: out_proj_half(1, 7, 0))
        add_slot(1, 3, 1, lambda: out_proj_half(1, 7, 1))
        for j in range(2, 16):
            add_slot(1, 3, j, transpose_dummy)

        # ---- attention ----
        attention_all()

        # ---- tail: transposes of (qg1, pair1) + out-proj of qg1 ----
        for qt in range(4):
            transpose_at(1, 1, qt, attn_tiles[(1, 1)][qt])
        for mt in range(8):
            out_proj_half(2, mt, 0, tail=True)
            out_proj_half(2, mt, 1, tail=True)
            if mt < 4:
                transpose_at(1, 1, 4 + mt, attn_tiles[(1, 1)][4 + mt])
        for mt in range(8):
            out_proj_half(3, mt, 0, tail=True)
            out_proj_half(3, mt, 1, tail=True)

    nc.compile()
    return nc


def make_in_maps(x, Wq, bq, Wk, bk, Wv, bv, Wo):
    """Shard full inputs into 8 per-core input maps."""
    import ml_dtypes
    BF = ml_dtypes.bfloat16
    scale = np.float32(1.0 / np.sqrt(DH))
    xT = [np.ascontiguousarray(x[b].T).astype(BF) for b in range(2)]
    ident = np.eye(128, dtype=np.float32).astype(BF)
    in_maps = []
    for c in range(8):
        b, g = c // 4, c % 4
        sl = slice(C * g, C * (g + 1))
        brow_g = np.concatenate([bq[sl] * scale, bk[sl]])[None, :]
        in_maps.append({
            "xT": xT[b],
            "wq": (np.ascontiguousarray(Wq[:, sl]) * scale).astype(BF),
            "wk": np.ascontiguousarray(Wk[:, sl]).astype(BF),
            "wv": np.ascontiguousarray(Wv[:, sl]).astype(BF),
            "wo": np.ascontiguousarray(Wo[sl, :]).astype(BF),
            "brow": np.ascontiguousarray(brow_g).astype(BF),
            "ident": ident,
        })
    return in_maps


def kernel(x, Wq, bq, Wk, bk, Wv, bv, Wo, bo):
    if os.environ.get("JAX_PLATFORMS") and \
            "axon" not in os.environ["JAX_PLATFORMS"]:
        os.environ.pop("JAX_PLATFORMS")
    trace = bool(os.environ.get("KERNEL_TRACE"))
    if trace:
        _install_ntff_shim()
    from concourse import bass_utils

    x = np.asarray(x, dtype=np.float32)
    in_maps = make_in_maps(
        x, np.asarray(Wq), np.asarray(bq), np.asarray(Wk), np.asarray(bk),
        np.asarray(Wv), np.asarray(bv), np.asarray(Wo))

    if "nc" not in _CACHE:
        _CACHE["nc"] = build_nc()
    res = bass_utils.run_bass_kernel_spmd(
        _CACHE["nc"], in_maps, core_ids=list(range(8)), trace=trace)
    _CACHE["exec_time_ns"] = res.exec_time_ns

    # softmax rows sum to 1, so the v-bias contributes exactly bv @ Wo
    bo_eff = (np.asarray(bo, dtype=np.float32)
              + np.asarray(bv, dtype=np.float32)
              @ np.asarray(Wo, dtype=np.float32))
    out = np.empty((2, S, D), dtype=np.float32)
    for b in range(2):
        acc = res.results[4 * b]["yT"].astype(np.float32)
        for g in range(1, 4):
            acc += res.results[4 * b + g]["yT"].astype(np.float32)
        out[b] = acc.T + bo_eff
    return out


# revision 25
# speedup vs baseline: 1.1472x; 1.1472x over previous
"""Multi-head attention (B=2, S=2048, D=1024, H=16) on 8 TRN2 NeuronCores, v3.

Sharding: data-parallel over batch (2) x tensor-parallel over head groups
(4 groups of 4 heads).  Core c = (b = c // 4, g = c % 4).

v3 design (HAM-warm dense-PE schedule):
  - All q/k/v projections in bf16; qT/kT stored PER HEAD with the 64 dh rows
    duplicated to partitions 64-127 (dup via SBUF->SBUF DMA) so the two
    512-wide q-blocks of a scores tile run CONCURRENTLY in different PE
    row-groups (tile_position auto-derived from base partitions).
  - Projections are emitted as ~1024-cycle quarter-units and spread through
    the attention j-slots so the PE never idles -> HAM stays at K=8/8
    (2.4 GHz).  Empty late slots get dummy transposes to hold the clock.
  - Act engine runs exp back-to-back ([128,1024] per (head, kt)); it is the
    steady-state bottleneck (~1.1us/instr).
  - PV in [q, dh] orientation: lhsT = pt chunk [k,128q], rhs = v_ext [k,65]
    (64 v cols + ones col -> denominators land in pv col 64).
  - normalize = DVE reciprocal + per-partition tensor_scalar_mul; PE
    transpose puts normalized attn into aT [d, q] (+v bias folded in).
  - yT written as bf16 (halves output DMA); host accumulates in fp32.
"""

import os
import sys
import types
from contextlib import ExitStack

import numpy as np

D = 1024
S = 2048
C = 256          # head cols per core (4 heads x 64)
DH = 64
NH = 4           # heads per core
QG = 1024        # q-group width
NQG = S // QG    # 2
NST = S // 128   # 16 seq tiles
NSB = QG // 128  # 8 q-subtiles per group

_CACHE = {}


def _install_ntff_shim():
    try:
        import antenv.axon_hooks  # noqa: F401
        return
    except ImportError:
        pass
    try:
        from trn_agent_boot.trn_boot import _ntff_profile_via_ctypes
        hook = _ntff_profile_via_ctypes('/opt/axon/libaxon_pjrt.so')
    except Exception:
        hook = None
    mod = types.ModuleType('antenv.axon_hooks')
    mod.get_axon_ntff_profile_hook = lambda: hook
    mod.set_axon_ntff_profile_hook = lambda h: None
    sys.modules['antenv.axon_hooks'] = mod


def build_nc():
    import concourse.bacc as bacc
    import concourse.mybir as mybir
    import concourse.tile as tile
    from concourse.bass import ts, ds

    F32 = mybir.dt.float32
    F32R = mybir.dt.float32r
    BF16 = mybir.dt.bfloat16
    ACT = mybir.ActivationFunctionType

    nc = bacc.Bacc("TRN2", target_bir_lowering=False, debug=False)
    xT = nc.dram_tensor("xT", [D, S], BF16, kind="ExternalInput")
    wq = nc.dram_tensor("wq", [D, C], BF16, kind="ExternalInput")
    wk = nc.dram_tensor("wk", [D, C], BF16, kind="ExternalInput")
    wv = nc.dram_tensor("wv", [D, C], BF16, kind="ExternalInput")
    wo = nc.dram_tensor("wo", [C, D], BF16, kind="ExternalInput")
    brow = nc.dram_tensor("brow", [1, 512], BF16, kind="ExternalInput")
    ident = nc.dram_tensor("ident", [128, 128], BF16, kind="ExternalInput")
    yT = nc.dram_tensor("yT", [D, S], BF16, kind="ExternalOutput")

    with tile.TileContext(nc) as tc, ExitStack() as ctx:
        consts = ctx.enter_context(tc.tile_pool(name="consts", bufs=1))
        sbw = ctx.enter_context(tc.tile_pool(name="weights", bufs=1))
        sbx = ctx.enter_context(tc.tile_pool(name="xsb", bufs=1))
        sbqkv = ctx.enter_context(tc.tile_pool(name="qkv", bufs=1))
        sbpt = ctx.enter_context(tc.tile_pool(name="ptp", bufs=3))
        sbat = ctx.enter_context(tc.tile_pool(name="atn", bufs=2))
        sbnrm = ctx.enter_context(tc.tile_pool(name="nrm", bufs=2))
        sby = ctx.enter_context(tc.tile_pool(name="ysb", bufs=4))
        sbtmp = ctx.enter_context(tc.tile_pool(name="tmpsb", bufs=2))
        sby0 = ctx.enter_context(tc.tile_pool(name="y0sb", bufs=1))
        # PSUM: sc 2x[128,1024] = 4 banks, pv 1x(2x[128,260]) = 2 banks,
        #       tr 1x[128,128] = 1 bank, yp 1x[128,512] = 1 bank -> 8 total
        scp = ctx.enter_context(tc.tile_pool(name="psc", bufs=2, space="PSUM"))
        pvp = ctx.enter_context(tc.tile_pool(name="ppv", bufs=1, space="PSUM"))
        trp = ctx.enter_context(tc.tile_pool(name="ptr", bufs=1, space="PSUM"))
        ypp = ctx.enter_context(tc.tile_pool(name="pyp", bufs=1, space="PSUM"))

        # ---- constants ----
        brow_sb = consts.tile([1, 512], BF16, tag="brow", name="brow_sb")
        nc.sync.dma_start(brow_sb[:], brow[:, :])
        ones_sb = consts.tile([1, 512], BF16, tag="ones", name="ones_sb")
        nc.gpsimd.memset(ones_sb[:], 1.0)
        id_sb = consts.tile([128, 128], BF16, tag="ident", name="id_sb")
        nc.sync.dma_start(id_sb[:], ident[:, :])

        # ---- input DMAs (ordered: wk, wv, x nb0, x nb1, wq, x nb2/3, wo) --
        wk_sb = [sbw.tile([128, C], BF16, tag=f"wk{i}", name=f"wk{i}")
                 for i in range(8)]
        wv_sb = [sbw.tile([128, C], BF16, tag=f"wv{i}", name=f"wv{i}")
                 for i in range(8)]
        wq_sb = [sbw.tile([128, C], BF16, tag=f"wq{i}", name=f"wq{i}")
                 for i in range(8)]
        xt_sb = [sbx.tile([128, S], BF16, tag=f"xt{i}", name=f"xt{i}")
                 for i in range(8)]
        for i in range(8):
            nc.sync.dma_start(wk_sb[i][:], wk[ts(i, 128), :])
        for i in range(8):
            nc.sync.dma_start(xt_sb[i][:, ts(0, 512)],
                              xT[ts(i, 128), ts(0, 512)])
        for i in range(8):
            nc.sync.dma_start(wq_sb[i][:], wq[ts(i, 128), :])
        for i in range(8):
            nc.sync.dma_start(xt_sb[i][:, ts(1, 512)],
                              xT[ts(i, 128), ts(1, 512)])
        for i in range(8):
            nc.sync.dma_start(wv_sb[i][:], wv[ts(i, 128), :])
        for nb in range(2, 4):
            for i in range(8):
                nc.sync.dma_start(xt_sb[i][:, ts(nb, 512)],
                                  xT[ts(i, 128), ts(nb, 512)])
        wo_sb = []
        for i in range(2):
            t = sbw.tile([128, D], BF16, tag=f"wo{i}", name=f"wo{i}")
            nc.sync.dma_start(t[:], wo[ts(i, 128), :])
            wo_sb.append(t)

        # ---- persistent activations ----
        # per-head q/k, dh rows duplicated into partitions 64-127
        qTd_sb = [sbqkv.tile([128, S], BF16, tag=f"qTd{h}", name=f"qTd{h}")
                  for h in range(NH)]
        kTd_sb = [sbqkv.tile([128, S], BF16, tag=f"kTd{h}", name=f"kTd{h}")
                  for h in range(NH)]
        v_sb = [sbqkv.tile([128, NH * 65], BF16, tag=f"v{i}", name=f"v{i}")
                for i in range(NST)]
        aT_sb = [sbqkv.tile([128, S], BF16, tag=f"aT{i}", name=f"aT{i}")
                 for i in range(2)]

        # ones columns of v_ext (col 65h+64 = 1.0)
        for st in range(NST):
            v3 = v_sb[st][:].rearrange("p (h e) -> p h e", e=65)
            nc.gpsimd.memset(v3[:, :, 64:65], 1.0)

        # ---- projection quarter-units (~1024 PE cycles each) ----
        open_pj = {}

        def proj_unit(which, mt, nb, u, pool=None, fast_dup=False):
            """2 of the 8 k-tile matmuls of one [128,512] q/k proj stripe;
            u==3 adds the bias via a K=1 ones-row matmul, casts to bf16
            once on DVE, then sprays the per-head row-dup copies via DMA."""
            w_sb, dsts, bcol = {
                "q": (wq_sb, qTd_sb, 0), "k": (wk_sb, kTd_sb, 256)}[which]
            key = (which, mt, nb)
            if u == 0:
                p = pool if pool is not None else ypp
                open_pj[key] = p.tile([128, 512], F32,
                                      tag="sc" if p is scp else "yp",
                                      name=f"pj_{which}{mt}{nb}")
            pj = open_pj[key]
            for kt in range(2 * u, 2 * u + 2):
                nc.tensor.matmul(
                    pj[:],
                    lhsT=w_sb[kt][:, ts(mt, 128)],
                    rhs=xt_sb[kt][:, ts(nb, 512)],
                    start=(kt == 0), stop=False,
                )
            if u == 3:
                nc.tensor.matmul(
                    pj[:],
                    lhsT=brow_sb[0:1, ds(bcol + 128 * mt, 128)],
                    rhs=ones_sb[0:1, :],
                    start=False, stop=True,
                )
                tmp = sbtmp.tile([128, 512], BF16, tag="pt16",
                                 name=f"pt16_{which}{mt}{nb}")
                nc.vector.tensor_copy(tmp[:], pj[:])
                eng = nc.scalar if fast_dup else nc.sync
                for hh in range(2):
                    dst = dsts[2 * mt + hh]
                    for rep in range(2):
                        eng.dma_start(
                            dst[64 * rep:64 * rep + 64, ts(nb, 512)],
                            tmp[64 * hh:64 * hh + 64, :])
                del open_pj[key]

        open_vp = {}

        def proj_v_half(st, half, pool=None):
            """half a seq-tile of v projection (4 of 8 k-tiles)."""
            if half == 0:
                p = pool if pool is not None else ypp
                open_vp[st] = p.tile([128, C], F32,
                                     tag="sc" if p is scp else "yp",
                                     name=f"vp{st}")
            vp = open_vp[st]
            for kt in range(4 * half, 4 * half + 4):
                nc.tensor.matmul(
                    vp[:],
                    lhsT=xt_sb[kt][:, ts(st, 128)],
                    rhs=wv_sb[kt][:],
                    start=(kt == 0), stop=(kt == 7),
                )
            if half == 1:
                v3 = v_sb[st][:].rearrange("p (h e) -> p h e", e=65)
                nc.vector.tensor_copy(
                    v3[:, :, 0:64],
                    vp[:].rearrange("p (h e) -> p h e", e=64))
                del open_vp[st]

        def transpose_at(pair, qg, qt, at_tile):
            """attn [128q,128d] -> aT[pair][...] via PE transpose + copy.
            v-bias is folded into the host-side output bias (softmax rows
            sum to 1, so attn@(v+bv) = attn@v + bv)."""
            tr = trp.tile([128, 128], BF16, tag="tr", name="tr")
            nc.tensor.transpose(tr[:], at_tile[:], id_sb[:])
            nc.vector.tensor_copy(
                aT_sb[pair][:, ds(qg * QG + qt * 128, 128)], tr[:])

        def transpose_dummy():
            """PE filler to keep the HAM clock-gate open in empty slots."""
            tr = trp.tile([128, 128], BF16, tag="tr", name="trd")
            nc.tensor.transpose(tr[:], id_sb[:], id_sb[:])

        # qg1 out-proj is split: pair0 partial computed early (hidden in
        # (1,3) slots, stashed in SBUF), pair1 matmul + add + store at tail.
        y0_sb = {}

        def out_proj_p0_store(nb, mt):
            yp = ypp.tile([128, 512], F32, tag="yp", name=f"y0p{nb}{mt}")
            nc.tensor.matmul(
                yp[:], lhsT=wo_sb[0][:, ts(mt, 128)],
                rhs=aT_sb[0][:, ts(nb, 512)], start=True, stop=True)
            t = sby0.tile([128, 512], F32, tag=f"y0_{nb}_{mt}",
                          name=f"y0_{nb}_{mt}")
            nc.vector.tensor_copy(t[:], yp[:])
            y0_sb[(nb, mt)] = t

        def out_proj_p1_add(nb, mt):
            yp = scp.tile([128, 512], F32, tag="sc", name=f"y1p{nb}{mt}")
            nc.tensor.matmul(
                yp[:], lhsT=wo_sb[1][:, ts(mt, 128)],
                rhs=aT_sb[1][:, ts(nb, 512)], start=True, stop=True)
            yt = sby.tile([128, 512], BF16, tag="yt", name="yt")
            nc.vector.tensor_add(yt[:], yp[:], y0_sb[(nb, mt)][:])
            nc.scalar.dma_start(yT[ts(mt, 128), ts(nb, 512)], yt[:])

        open_yp = {}

        def out_proj_half(nb, mt, p, tail=False):
            """one of the two accumulation matmuls of an out-proj stripe."""
            key = (nb, mt)
            if p == 0:
                pool = scp if tail else ypp
                open_yp[key] = pool.tile([128, 512], F32,
                                         tag="sc" if tail else "yp",
                                         name=f"yp{nb}{mt}")
            yp = open_yp[key]
            nc.tensor.matmul(
                yp[:],
                lhsT=wo_sb[p][:, ts(mt, 128)],
                rhs=aT_sb[p][:, ts(nb, 512)],
                start=(p == 0), stop=(p == 1),
            )
            if p == 1:
                yt = sby.tile([128, 512], BF16, tag="yt", name="yt")
                nc.vector.tensor_copy(yt[:], yp[:])
                eng = nc.scalar if tail else nc.sync
                eng.dma_start(yT[ts(mt, 128), ts(nb, 512)], yt[:])
                del open_yp[key]

        # ---- interleave schedule ----
        slots = {(qg, h): {} for qg in range(NQG) for h in range(NH)}

        def add_slot(qg, h, kt, fn):
            slots[(qg, h)].setdefault(kt, []).append(fn)

        # per-(qg,pair) attn tiles, filled by norm, consumed by transpose
        attn_tiles = {}

        def norm_pair_writes(qg, h, pva, pvb):
            pair = h // 2
            if (qg, pair) not in attn_tiles:
                attn_tiles[(qg, pair)] = [
                    sbat.tile([128, 128], BF16, tag=f"at{qt}", name=f"at{qt}")
                    for qt in range(NSB)]
            tiles = attn_tiles[(qg, pair)]
            col = 64 * (h % 2)
            pa3 = pva[:].rearrange("p (s e) -> p s e", e=65)
            pb3 = pvb[:].rearrange("p (s e) -> p s e", e=65)
            recip = sbnrm.tile([128, 8], F32, tag="rc", name="rc")
            nc.vector.reciprocal(recip[:, 0:4], pa3[:, :, 64])
            nc.vector.reciprocal(recip[:, 4:8], pb3[:, :, 64])
            for qs in range(NSB):
                src3 = pa3 if qs < 4 else pb3
                nc.vector.tensor_scalar_mul(
                    tiles[qs][:, col:col + 64],
                    src3[:, qs % 4, 0:64],
                    recip[:, qs:qs + 1])

        def attention_all():
            """single software pipeline over all (qg, h, kt): iteration t
            emits exp(t-1) FIRST (the act engine's coalesced PE-semaphore
            threshold then only covers work finished a full period ago),
            then scores(t), slot fillers, pv(t-2).  Flattening across head
            boundaries removes per-head pipeline drain/refill bubbles."""
            T = NQG * NH * NST
            state, pts, scs = {}, {}, {}

            def hq(t):
                head = t // NST
                return head // NH, head % NH, t % NST

            for t in range(T + 2):
                if 1 <= t <= T:
                    pt = sbpt.tile([128, QG], BF16, tag="pt", name="pt")
                    pts[t - 1] = pt
                    nc.scalar.activation(pt[:], scs.pop(t - 1)[:], ACT.Exp)
                if t < T:
                    qg, h, j = hq(t)
                    ktd, qtd = kTd_sb[h], qTd_sb[h]
                    sc = scp.tile([128, QG], F32, tag="sc", name="sc_at")
                    scs[t] = sc
                    # two q-blocks in different PE row groups -> concurrent
                    for qb in range(2):
                        rg = 64 * qb
                        nc.tensor.matmul(
                            sc[:, ts(qb, 512)],
                            lhsT=ktd[rg:rg + 64, ts(j, 128)],
                            rhs=qtd[rg:rg + 64,
                                    ds(qg * QG + qb * 512, 512)],
                            start=True, stop=True,
                        )
                    for fn in slots[(qg, h)].get(j, ()):
                        fn()
                t2 = t - 2
                if t2 >= 0:
                    qg2, h2, j2 = hq(t2)
                    if j2 == 0:
                        state[(qg2, h2)] = (
                            pvp.tile([128, 4 * 65], F32, tag="pva",
                                     name="pva"),
                            pvp.tile([128, 4 * 65], F32, tag="pvb",
                                     name="pvb"))
                    pva, pvb = state[(qg2, h2)]
                    ptt = pts.pop(t2)
                    for qs in range(NSB):
                        pvt = pva if qs < 4 else pvb
                        nc.tensor.matmul(
                            pvt[:, ds((qs % 4) * 65, 65)],
                            lhsT=ptt[:, ts(qs, 128)],
                            rhs=v_sb[j2][:, ds(65 * h2, 65)],
                            start=(j2 == 0 and qs % 4 == 0),
                            stop=(j2 == NST - 1 and qs % 4 == 3),
                        )
                    if j2 == NST - 1:
                        norm_pair_writes(qg2, h2, pva, pvb)
                        del state[(qg2, h2)]

        # ---- HAM warm-up: PE dummies chained on x-stripe DMA arrival
        # keep the clock-gate activity window busy through the DMA-bound
        # lead-in so projections run at 2.4 GHz as soon as data lands.
        def warm_dummy(i, col):
            tr = trp.tile([128, 128], BF16, tag="tr", name="trw")
            nc.tensor.transpose(tr[:], xt_sb[i][:, ds(col, 128)], id_sb[:])

        for i in range(8):
            for c in range(2):
                warm_dummy(i, 256 * c)
        for i in range(8):
            for c in range(2):
                warm_dummy(i, 512 + 256 * c)

        # ---- lead-in: only what scores(0,0) j0 needs ----
        # k mt0 nb0 (kT head0 cols 0-512) + q mt0 nb0+nb1 (qT head0, qg0).
        for u in range(4):
            proj_unit("k", 0, 0, u, pool=scp, fast_dup=True)
        for u in range(4):
            proj_unit("q", 0, 0, u, pool=ypp, fast_dup=True)
        for u in range(4):
            proj_unit("q", 0, 1, u, pool=scp, fast_dup=True)

        # ---- slot fillers ----
        # unit helpers for slot lambdas
        def k_unit(mt, nb, u):
            return lambda: proj_unit("k", mt, nb, u)

        def q_unit(mt, nb, u):
            return lambda: proj_unit("q", mt, nb, u)

        def v_full(st):
            return [lambda st=st: proj_v_half(st, 0),
                    lambda st=st: proj_v_half(st, 1)]

        # (0,0): k mt0 catch-up (nb1 by j4, nb2 by j8, nb3 by j12),
        # v st0-15 just in time, q mt1 nb0/nb1.  One psum group per slot
        # boundary (groups never interleave).
        plan00 = {
            0: [k_unit(0, 1, u) for u in range(4)],
            1: v_full(0) + v_full(1),
            2: [k_unit(0, 2, u) for u in range(4)],
            3: v_full(2) + v_full(3),
            4: v_full(4) + v_full(5),
            5: [k_unit(0, 3, u) for u in range(4)],
            6: v_full(6) + v_full(7),
            7: v_full(8) + v_full(9),
            8: v_full(10),
            9: v_full(11),
            10: v_full(12),
            11: v_full(13),
            12: v_full(14),
            13: v_full(15),
            14: [transpose_dummy],
            15: [transpose_dummy],
        }
        for j, fns in plan00.items():
            for fn in fns:
                add_slot(0, 0, j, fn)
        # (0,1): k mt1 all four nb + q mt1 nb0/nb1 (needed by (0,2))
        for j in range(16):
            nb, u = j // 4, j % 4
            add_slot(0, 1, j, k_unit(1, nb, u))
        for j in range(8):
            nb, u = j // 4, j % 4
            add_slot(0, 1, 8 + j, q_unit(1, nb, u))
        # NOTE: norm of head (qg,h) is emitted at global iteration
        # 16*head+17 = next head's j1, so transpose slots start at j>=2.
        # (0,2): transposes pair0 qg0 on j2-9; q mt0 nb2/nb3 on j10-15+
        for qt in range(NSB):
            add_slot(0, 2, 4 + qt, (lambda qt=qt:
                     transpose_at(0, 0, qt, attn_tiles[(0, 0)][qt])))
        for j in range(6):
            nb, u = 2 + j // 4, j % 4
            add_slot(0, 2, 10 + j, q_unit(0, nb, u))
        # (0,3): q mt0 nb3 tail, q mt1 nb2/nb3, dummies
        add_slot(0, 3, 0, q_unit(0, 3, 2))
        add_slot(0, 3, 1, q_unit(0, 3, 3))
        for j in range(8):
            nb, u = 2 + j // 4, j % 4
            add_slot(0, 3, 2 + j, q_unit(1, nb, u))
        for j in range(10, 16):
            add_slot(0, 3, j, transpose_dummy)
        # (1,0): transposes pair1 qg0 on j2-9; out-proj nb0 mt0-2 j10-15
        for qt in range(NSB):
            add_slot(1, 0, 4 + qt, (lambda qt=qt:
                     transpose_at(1, 0, qt, attn_tiles[(0, 1)][qt])))
        for j in range(6):
            mt, p = j // 2, j % 2
            add_slot(1, 0, 10 + j, (lambda mt=mt, p=p:
                                    out_proj_half(0, mt, p)))
        # (1,1): out-proj nb0 mt3-7, nb1 mt0-2
        for j in range(10):
            mt, p = 3 + j // 2, j % 2
            add_slot(1, 1, j, (lambda mt=mt, p=p:
                               out_proj_half(0, mt, p)))
        for j in range(6):
            mt, p = j // 2, j % 2
            add_slot(1, 1, 10 + j, (lambda mt=mt, p=p:
                                    out_proj_half(1, mt, p)))
        # (1,2): out-proj nb1 mt3; transposes pair0 qg1 j2-9; nb1 mt4-7
        add_slot(1, 2, 0, lambda: out_proj_half(1, 3, 0))
        add_slot(1, 2, 1, lambda: out_proj_half(1, 3, 1))
        for qt in range(NSB):
            add_slot(1, 2, 4 + qt, (lambda qt=qt:
                     transpose_at(0, 1, qt, attn_tiles[(1, 0)][qt])))
        for j in range(6):
            mt, p = 4 + j // 2, j % 2
            add_slot(1, 2, 10 + j, (lambda mt=mt, p=p:
                                    out_proj_half(1, mt, p)))
        # (1,3): out-proj nb1 mt7; qg1 pair0 out-proj partials
        add_slot(1, 3, 0, lambda: out_proj_half(1, 7, 0))
        add_slot(1, 3, 1, lambda: out_proj_half(1, 7, 1))
        for j in range(14):
            nb, mt = 2 + j // 8, j % 8
            add_slot(1, 3, 2 + j, (lambda nb=nb, mt=mt:
                                   out_proj_p0_store(nb, mt)))
        add_slot(1, 3, 15, lambda: out_proj_p0_store(3, 6))
        add_slot(1, 3, 15, lambda: out_proj_p0_store(3, 7))

        # ---- attention ----
        attention_all()

        # ---- tail: transposes of (qg1, pair1) + pair1 out-proj + add ----
        for qt in range(4):
            transpose_at(1, 1, qt, attn_tiles[(1, 1)][qt])
        for mt in range(8):
            out_proj_p1_add(2, mt)
            if mt < 4:
                transpose_at(1, 1, 4 + mt, attn_tiles[(1, 1)][4 + mt])
        for mt in range(8):
            out_proj_p1_add(3, mt)

    nc.compile()
    return nc


def make_in_maps(x, Wq, bq, Wk, bk, Wv, bv, Wo):
    """Shard full inputs into 8 per-core input maps."""
    import ml_dtypes
    BF = ml_dtypes.bfloat16
    scale = np.float32(1.0 / np.sqrt(DH))
    xT = [np.ascontiguousarray(x[b].T).astype(BF) for b in range(2)]
    ident = np.eye(128, dtype=np.float32).astype(BF)
    in_maps = []
    for c in range(8):
        b, g = c // 4, c % 4
        sl = slice(C * g, C * (g + 1))
        brow_g = np.concatenate([bq[sl] * scale, bk[sl]])[None, :]
        in_maps.append({
            "xT": xT[b],
            "wq": (np.ascontiguousarray(Wq[:, sl]) * scale).astype(BF),
            "wk": np.ascontiguousarray(Wk[:, sl]).astype(BF),
            "wv": np.ascontiguousarray(Wv[:, sl]).astype(BF),
            "wo": np.ascontiguousarray(Wo[sl, :]).astype(BF),
            "brow": np.ascontiguousarray(brow_g).astype(BF),
            "ident": ident,
        })
    return in_maps


def kernel(x, Wq, bq, Wk, bk, Wv, bv, Wo, bo):
    if os.environ.get("JAX_PLATFORMS") and \
            "axon" not in os.environ["JAX_PLATFORMS"]:
        os.environ.pop("JAX_PLATFORMS")
    trace = bool(os.environ.get("KERNEL_TRACE"))
    if trace:
        _install_ntff_shim()
    from concourse import bass_utils

    x = np.asarray(x, dtype=np.float32)
    in_maps = make_in_maps(
        x, np.asarray(Wq), np.asarray(bq), np.asarray(Wk), np.asarray(bk),
        np.asarray(Wv), np.asarray(bv), np.asarray(Wo))

    if "nc" not in _CACHE:
        _CACHE["nc"] = build_nc()
    res = bass_utils.run_bass_kernel_spmd(
        _CACHE["nc"], in_maps, core_ids=list(range(8)), trace=trace)
    _CACHE["exec_time_ns"] = res.exec_time_ns

    # softmax rows sum to 1, so the v-bias contributes exactly bv @ Wo
    bo_eff = (np.asarray(bo, dtype=np.float32)
              + np.asarray(bv, dtype=np.float32)
              @ np.asarray(Wo, dtype=np.float32))
    out = np.empty((2, S, D), dtype=np.float32)
    for b in range(2):
        acc = res.results[4 * b]["yT"].astype(np.float32)
        for g in range(1, 4):
            acc += res.results[4 * b + g]["yT"].astype(np.float32)
        out[b] = acc.T + bo_eff
    return out


# revision 26
# speedup vs baseline: 1.1698x; 1.0197x over previous
"""Multi-head attention (B=2, S=2048, D=1024, H=16) on 8 TRN2 NeuronCores, v3.

Sharding: data-parallel over batch (2) x tensor-parallel over head groups
(4 groups of 4 heads).  Core c = (b = c // 4, g = c % 4).

v3 design (HAM-warm dense-PE schedule):
  - All q/k/v projections in bf16; qT/kT stored PER HEAD with the 64 dh rows
    duplicated to partitions 64-127 (dup via SBUF->SBUF DMA) so the two
    512-wide q-blocks of a scores tile run CONCURRENTLY in different PE
    row-groups (tile_position auto-derived from base partitions).
  - Projections are emitted as ~1024-cycle quarter-units and spread through
    the attention j-slots so the PE never idles -> HAM stays at K=8/8
    (2.4 GHz).  Empty late slots get dummy transposes to hold the clock.
  - Act engine runs exp back-to-back ([128,1024] per (head, kt)); it is the
    steady-state bottleneck (~1.1us/instr).
  - PV in [q, dh] orientation: lhsT = pt chunk [k,128q], rhs = v_ext [k,65]
    (64 v cols + ones col -> denominators land in pv col 64).
  - normalize = DVE reciprocal + per-partition tensor_scalar_mul; PE
    transpose puts normalized attn into aT [d, q] (+v bias folded in).
  - yT written as bf16 (halves output DMA); host accumulates in fp32.
"""

import os
import sys
import types
from contextlib import ExitStack

import numpy as np

D = 1024
S = 2048
C = 256          # head cols per core (4 heads x 64)
DH = 64
NH = 4           # heads per core
QG = 1024        # q-group width
NQG = S // QG    # 2
NST = S // 128   # 16 seq tiles
NSB = QG // 128  # 8 q-subtiles per group

_CACHE = {}


def _install_ntff_shim():
    try:
        import antenv.axon_hooks  # noqa: F401
        return
    except ImportError:
        pass
    try:
        from trn_agent_boot.trn_boot import _ntff_profile_via_ctypes
        hook = _ntff_profile_via_ctypes('/opt/axon/libaxon_pjrt.so')
    except Exception:
        hook = None
    mod = types.ModuleType('antenv.axon_hooks')
    mod.get_axon_ntff_profile_hook = lambda: hook
    mod.set_axon_ntff_profile_hook = lambda h: None
    sys.modules['antenv.axon_hooks'] = mod


def build_nc():
    import concourse.bacc as bacc
    import concourse.mybir as mybir
    import concourse.tile as tile
    from concourse.bass import ts, ds

    F32 = mybir.dt.float32
    F32R = mybir.dt.float32r
    BF16 = mybir.dt.bfloat16
    ACT = mybir.ActivationFunctionType

    nc = bacc.Bacc("TRN2", target_bir_lowering=False, debug=False)
    xT = nc.dram_tensor("xT", [D, S], BF16, kind="ExternalInput")
    wq = nc.dram_tensor("wq", [D, C], BF16, kind="ExternalInput")
    wk = nc.dram_tensor("wk", [D, C], BF16, kind="ExternalInput")
    wv = nc.dram_tensor("wv", [D, C], BF16, kind="ExternalInput")
    wo = nc.dram_tensor("wo", [C, D], BF16, kind="ExternalInput")
    brow = nc.dram_tensor("brow", [1, 512], BF16, kind="ExternalInput")
    ident = nc.dram_tensor("ident", [128, 128], BF16, kind="ExternalInput")
    yT = nc.dram_tensor("yT", [D, S], BF16, kind="ExternalOutput")

    with tile.TileContext(nc) as tc, ExitStack() as ctx:
        consts = ctx.enter_context(tc.tile_pool(name="consts", bufs=1))
        sbw = ctx.enter_context(tc.tile_pool(name="weights", bufs=1))
        sbx = ctx.enter_context(tc.tile_pool(name="xsb", bufs=1))
        sbqkv = ctx.enter_context(tc.tile_pool(name="qkv", bufs=1))
        sbpt = ctx.enter_context(tc.tile_pool(name="ptp", bufs=3))
        sbat = ctx.enter_context(tc.tile_pool(name="atn", bufs=2))
        sbnrm = ctx.enter_context(tc.tile_pool(name="nrm", bufs=2))
        sby = ctx.enter_context(tc.tile_pool(name="ysb", bufs=4))
        sbtmp = ctx.enter_context(tc.tile_pool(name="tmpsb", bufs=2))
        sby0 = ctx.enter_context(tc.tile_pool(name="y0sb", bufs=1))
        # PSUM: sc 2x[128,1024] = 4 banks, pv 1x(2x[128,260]) = 2 banks,
        #       tr 1x[128,128] = 1 bank, yp 1x[128,512] = 1 bank -> 8 total
        scp = ctx.enter_context(tc.tile_pool(name="psc", bufs=2, space="PSUM"))
        pvp = ctx.enter_context(tc.tile_pool(name="ppv", bufs=1, space="PSUM"))
        trp = ctx.enter_context(tc.tile_pool(name="ptr", bufs=1, space="PSUM"))
        ypp = ctx.enter_context(tc.tile_pool(name="pyp", bufs=1, space="PSUM"))

        # ---- constants ----
        brow_sb = consts.tile([1, 512], BF16, tag="brow", name="brow_sb")
        nc.sync.dma_start(brow_sb[:], brow[:, :])
        ones_sb = consts.tile([1, 512], BF16, tag="ones", name="ones_sb")
        nc.gpsimd.memset(ones_sb[:], 1.0)
        id_sb = consts.tile([128, 128], BF16, tag="ident", name="id_sb")
        nc.sync.dma_start(id_sb[:], ident[:, :])

        # ---- input DMAs (ordered: wk, wv, x nb0, x nb1, wq, x nb2/3, wo) --
        wk_sb = [sbw.tile([128, C], BF16, tag=f"wk{i}", name=f"wk{i}")
                 for i in range(8)]
        wv_sb = [sbw.tile([128, C], BF16, tag=f"wv{i}", name=f"wv{i}")
                 for i in range(8)]
        wq_sb = [sbw.tile([128, C], BF16, tag=f"wq{i}", name=f"wq{i}")
                 for i in range(8)]
        xt_sb = [sbx.tile([128, S], BF16, tag=f"xt{i}", name=f"xt{i}")
                 for i in range(8)]
        for i in range(8):
            nc.sync.dma_start(wk_sb[i][:], wk[ts(i, 128), :])
        for i in range(8):
            nc.sync.dma_start(xt_sb[i][:, ts(0, 512)],
                              xT[ts(i, 128), ts(0, 512)])
        for i in range(8):
            nc.sync.dma_start(wq_sb[i][:], wq[ts(i, 128), :])
        for i in range(8):
            nc.sync.dma_start(xt_sb[i][:, ts(1, 512)],
                              xT[ts(i, 128), ts(1, 512)])
        for i in range(8):
            nc.sync.dma_start(wv_sb[i][:], wv[ts(i, 128), :])
        for nb in range(2, 4):
            for i in range(8):
                nc.sync.dma_start(xt_sb[i][:, ts(nb, 512)],
                                  xT[ts(i, 128), ts(nb, 512)])
        wo_sb = []
        for i in range(2):
            t = sbw.tile([128, D], BF16, tag=f"wo{i}", name=f"wo{i}")
            nc.sync.dma_start(t[:], wo[ts(i, 128), :])
            wo_sb.append(t)

        # ---- persistent activations ----
        # per-head q/k, dh rows duplicated into partitions 64-127
        qTd_sb = [sbqkv.tile([128, S], BF16, tag=f"qTd{h}", name=f"qTd{h}")
                  for h in range(NH)]
        kTd_sb = [sbqkv.tile([128, S], BF16, tag=f"kTd{h}", name=f"kTd{h}")
                  for h in range(NH)]
        v_sb = [sbqkv.tile([128, NH * 65], BF16, tag=f"v{i}", name=f"v{i}")
                for i in range(NST)]
        aT_sb = [sbqkv.tile([128, S], BF16, tag=f"aT{i}", name=f"aT{i}")
                 for i in range(2)]

        # ones columns of v_ext (col 65h+64 = 1.0)
        for st in range(NST):
            v3 = v_sb[st][:].rearrange("p (h e) -> p h e", e=65)
            nc.gpsimd.memset(v3[:, :, 64:65], 1.0)

        # ---- projection quarter-units (~1024 PE cycles each) ----
        open_pj = {}

        def proj_unit(which, mt, nb, u, pool=None, fast_dup=False):
            """2 of the 8 k-tile matmuls of one [128,512] q/k proj stripe;
            u==3 adds the bias via a K=1 ones-row matmul, casts to bf16
            once on DVE, then sprays the per-head row-dup copies via DMA."""
            w_sb, dsts, bcol = {
                "q": (wq_sb, qTd_sb, 0), "k": (wk_sb, kTd_sb, 256)}[which]
            key = (which, mt, nb)
            if u == 0:
                p = pool if pool is not None else ypp
                open_pj[key] = p.tile([128, 512], F32,
                                      tag="sc" if p is scp else "yp",
                                      name=f"pj_{which}{mt}{nb}")
            pj = open_pj[key]
            for kt in range(2 * u, 2 * u + 2):
                nc.tensor.matmul(
                    pj[:],
                    lhsT=w_sb[kt][:, ts(mt, 128)],
                    rhs=xt_sb[kt][:, ts(nb, 512)],
                    start=(kt == 0), stop=False,
                )
            if u == 3:
                nc.tensor.matmul(
                    pj[:],
                    lhsT=brow_sb[0:1, ds(bcol + 128 * mt, 128)],
                    rhs=ones_sb[0:1, :],
                    start=False, stop=True,
                )
                tmp = sbtmp.tile([128, 512], BF16, tag="pt16",
                                 name=f"pt16_{which}{mt}{nb}")
                nc.vector.tensor_copy(tmp[:], pj[:])
                eng = nc.scalar if fast_dup else nc.sync
                for hh in range(2):
                    dst = dsts[2 * mt + hh]
                    for rep in range(2):
                        eng.dma_start(
                            dst[64 * rep:64 * rep + 64, ts(nb, 512)],
                            tmp[64 * hh:64 * hh + 64, :])
                del open_pj[key]

        open_vp = {}

        def proj_v_half(st, half, pool=None):
            """half a seq-tile of v projection (4 of 8 k-tiles)."""
            if half == 0:
                p = pool if pool is not None else ypp
                open_vp[st] = p.tile([128, C], F32,
                                     tag="sc" if p is scp else "yp",
                                     name=f"vp{st}")
            vp = open_vp[st]
            for kt in range(4 * half, 4 * half + 4):
                nc.tensor.matmul(
                    vp[:],
                    lhsT=xt_sb[kt][:, ts(st, 128)],
                    rhs=wv_sb[kt][:],
                    start=(kt == 0), stop=(kt == 7),
                )
            if half == 1:
                v3 = v_sb[st][:].rearrange("p (h e) -> p h e", e=65)
                nc.vector.tensor_copy(
                    v3[:, :, 0:64],
                    vp[:].rearrange("p (h e) -> p h e", e=64))
                del open_vp[st]

        def transpose_at(pair, qg, qt, at_tile):
            """attn [128q,128d] -> aT[pair][...] via PE transpose + copy.
            v-bias is folded into the host-side output bias (softmax rows
            sum to 1, so attn@(v+bv) = attn@v + bv)."""
            tr = trp.tile([128, 128], BF16, tag="tr", name="tr")
            nc.tensor.transpose(tr[:], at_tile[:], id_sb[:])
            nc.vector.tensor_copy(
                aT_sb[pair][:, ds(qg * QG + qt * 128, 128)], tr[:])

        def transpose_dummy():
            """PE filler to keep the HAM clock-gate open in empty slots."""
            tr = trp.tile([128, 128], BF16, tag="tr", name="trd")
            nc.tensor.transpose(tr[:], id_sb[:], id_sb[:])

        # qg1 out-proj is split: pair0 partial computed early (hidden in
        # (1,3) slots, stashed in SBUF), pair1 matmul + add + store at tail.
        y0_sb = {}

        def out_proj_p0_store(nb, mt):
            yp = ypp.tile([128, 512], F32, tag="yp", name=f"y0p{nb}{mt}")
            nc.tensor.matmul(
                yp[:], lhsT=wo_sb[0][:, ts(mt, 128)],
                rhs=aT_sb[0][:, ts(nb, 512)], start=True, stop=True)
            t = sby0.tile([128, 512], F32, tag=f"y0_{nb}_{mt}",
                          name=f"y0_{nb}_{mt}")
            nc.vector.tensor_copy(t[:], yp[:])
            y0_sb[(nb, mt)] = t

        def out_proj_p1_add(nb, mt):
            yp = scp.tile([128, 512], F32, tag="sc", name=f"y1p{nb}{mt}")
            nc.tensor.matmul(
                yp[:], lhsT=wo_sb[1][:, ts(mt, 128)],
                rhs=aT_sb[1][:, ts(nb, 512)], start=True, stop=True)
            yt = sby.tile([128, 512], BF16, tag="yt", name="yt")
            nc.vector.tensor_add(yt[:], yp[:], y0_sb[(nb, mt)][:])
            nc.scalar.dma_start(yT[ts(mt, 128), ts(nb, 512)], yt[:])

        open_yp = {}

        def out_proj_half(nb, mt, p, tail=False):
            """one of the two accumulation matmuls of an out-proj stripe."""
            key = (nb, mt)
            if p == 0:
                pool = scp if tail else ypp
                open_yp[key] = pool.tile([128, 512], F32,
                                         tag="sc" if tail else "yp",
                                         name=f"yp{nb}{mt}")
            yp = open_yp[key]
            nc.tensor.matmul(
                yp[:],
                lhsT=wo_sb[p][:, ts(mt, 128)],
                rhs=aT_sb[p][:, ts(nb, 512)],
                start=(p == 0), stop=(p == 1),
            )
            if p == 1:
                yt = sby.tile([128, 512], BF16, tag="yt", name="yt")
                nc.vector.tensor_copy(yt[:], yp[:])
                eng = nc.scalar if tail else nc.sync
                eng.dma_start(yT[ts(mt, 128), ts(nb, 512)], yt[:])
                del open_yp[key]

        # ---- interleave schedule ----
        slots = {(qg, h): {} for qg in range(NQG) for h in range(NH)}

        def add_slot(qg, h, kt, fn):
            slots[(qg, h)].setdefault(kt, []).append(fn)

        # per-(qg,pair) attn tiles, filled by norm, consumed by transpose
        attn_tiles = {}

        def norm_pair_writes(qg, h, pva, pvb):
            pair = h // 2
            if (qg, pair) not in attn_tiles:
                attn_tiles[(qg, pair)] = [
                    sbat.tile([128, 128], BF16, tag=f"at{qt}", name=f"at{qt}")
                    for qt in range(NSB)]
            tiles = attn_tiles[(qg, pair)]
            col = 64 * (h % 2)
            pa3 = pva[:].rearrange("p (s e) -> p s e", e=65)
            pb3 = pvb[:].rearrange("p (s e) -> p s e", e=65)
            recip = sbnrm.tile([128, 8], F32, tag="rc", name="rc")
            nc.vector.reciprocal(recip[:, 0:4], pa3[:, :, 64])
            nc.vector.reciprocal(recip[:, 4:8], pb3[:, :, 64])
            for qs in range(NSB):
                src3 = pa3 if qs < 4 else pb3
                nc.vector.tensor_scalar_mul(
                    tiles[qs][:, col:col + 64],
                    src3[:, qs % 4, 0:64],
                    recip[:, qs:qs + 1])

        def attention_all():
            """single software pipeline over all (qg, h, kt): iteration t
            emits exp(t-1) FIRST (the act engine's coalesced PE-semaphore
            threshold then only covers work finished a full period ago),
            then scores(t), slot fillers, pv(t-2).  Flattening across head
            boundaries removes per-head pipeline drain/refill bubbles."""
            T = NQG * NH * NST
            state, pts, scs = {}, {}, {}

            def hq(t):
                head = t // NST
                return head // NH, head % NH, t % NST

            for t in range(T + 2):
                if 1 <= t <= T:
                    pt = sbpt.tile([128, QG], BF16, tag="pt", name="pt")
                    pts[t - 1] = pt
                    nc.scalar.activation(pt[:], scs.pop(t - 1)[:], ACT.Exp)
                if t < T:
                    qg, h, j = hq(t)
                    ktd, qtd = kTd_sb[h], qTd_sb[h]
                    sc = scp.tile([128, QG], F32, tag="sc", name="sc_at")
                    scs[t] = sc
                    # two q-blocks in different PE row groups -> concurrent
                    for qb in range(2):
                        rg = 64 * qb
                        nc.tensor.matmul(
                            sc[:, ts(qb, 512)],
                            lhsT=ktd[rg:rg + 64, ts(j, 128)],
                            rhs=qtd[rg:rg + 64,
                                    ds(qg * QG + qb * 512, 512)],
                            start=True, stop=True,
                        )
                    for fn in slots[(qg, h)].get(j, ()):
                        fn()
                t2 = t - 2
                if t2 >= 0:
                    qg2, h2, j2 = hq(t2)
                    if j2 == 0:
                        state[(qg2, h2)] = (
                            pvp.tile([128, 4 * 65], F32, tag="pva",
                                     name="pva"),
                            pvp.tile([128, 4 * 65], F32, tag="pvb",
                                     name="pvb"))
                    pva, pvb = state[(qg2, h2)]
                    ptt = pts.pop(t2)
                    for qs in range(NSB):
                        pvt = pva if qs < 4 else pvb
                        nc.tensor.matmul(
                            pvt[:, ds((qs % 4) * 65, 65)],
                            lhsT=ptt[:, ts(qs, 128)],
                            rhs=v_sb[j2][:, ds(65 * h2, 65)],
                            start=(j2 == 0 and qs % 4 == 0),
                            stop=(j2 == NST - 1 and qs % 4 == 3),
                        )
                    if j2 == NST - 1:
                        norm_pair_writes(qg2, h2, pva, pvb)
                        del state[(qg2, h2)]

        # ---- HAM warm-up: PE dummies chained on x-stripe DMA arrival
        # keep the clock-gate activity window busy through the DMA-bound
        # lead-in so projections run at 2.4 GHz as soon as data lands.
        def warm_dummy(i, col):
            tr = trp.tile([128, 128], BF16, tag="tr", name="trw")
            nc.tensor.transpose(tr[:], xt_sb[i][:, ds(col, 128)], id_sb[:])

        for i in range(8):
            for c in range(2):
                warm_dummy(i, 256 * c)

        # ---- lead-in: only what scores(0,0) j0 needs ----
        # k mt0 nb0 (kT head0 cols 0-512) + q mt0 nb0+nb1 (qT head0, qg0).
        for u in range(4):
            proj_unit("k", 0, 0, u, pool=scp, fast_dup=True)
        for u in range(2):
            proj_unit("q", 0, 0, u, pool=ypp, fast_dup=True)
        for i in range(0, 8, 2):
            warm_dummy(i, 512)
        for u in range(2, 4):
            proj_unit("q", 0, 0, u, pool=ypp, fast_dup=True)
        for i in range(1, 8, 2):
            warm_dummy(i, 512)
        for u in range(4):
            proj_unit("q", 0, 1, u, pool=scp, fast_dup=True)

        # ---- slot fillers ----
        # unit helpers for slot lambdas
        def k_unit(mt, nb, u):
            return lambda: proj_unit("k", mt, nb, u)

        def q_unit(mt, nb, u):
            return lambda: proj_unit("q", mt, nb, u)

        def v_full(st):
            return [lambda st=st: proj_v_half(st, 0),
                    lambda st=st: proj_v_half(st, 1)]

        # (0,0): k mt0 catch-up (nb1 by j4, nb2 by j8, nb3 by j12),
        # v st0-15 just in time, q mt1 nb0/nb1.  One psum group per slot
        # boundary (groups never interleave).
        plan00 = {
            0: [k_unit(0, 1, u) for u in range(4)],
            1: v_full(0) + v_full(1),
            2: [k_unit(0, 2, u) for u in range(4)],
            3: v_full(2) + v_full(3),
            4: v_full(4) + v_full(5),
            5: [k_unit(0, 3, u) for u in range(4)],
            6: v_full(6) + v_full(7),
            7: v_full(8) + v_full(9),
            8: v_full(10),
            9: v_full(11),
            10: v_full(12),
            11: v_full(13),
            12: v_full(14),
            13: v_full(15),
            14: [transpose_dummy],
            15: [transpose_dummy],
        }
        for j, fns in plan00.items():
            for fn in fns:
                add_slot(0, 0, j, fn)
        # (0,1): k mt1 all four nb + q mt1 nb0/nb1 (needed by (0,2))
        for j in range(16):
            nb, u = j // 4, j % 4
            add_slot(0, 1, j, k_unit(1, nb, u))
        for j in range(8):
            nb, u = j // 4, j % 4
            add_slot(0, 1, 8 + j, q_unit(1, nb, u))
        # NOTE: norm of head (qg,h) is emitted at global iteration
        # 16*head+17 = next head's j1, so transpose slots start at j>=2.
        # (0,2): transposes pair0 qg0 on j2-9; q mt0 nb2/nb3 on j10-15+
        for qt in range(NSB):
            add_slot(0, 2, 4 + qt, (lambda qt=qt:
                     transpose_at(0, 0, qt, attn_tiles[(0, 0)][qt])))
        for j in range(6):
            nb, u = 2 + j // 4, j % 4
            add_slot(0, 2, 10 + j, q_unit(0, nb, u))
        # (0,3): q mt0 nb3 tail, q mt1 nb2/nb3, dummies
        add_slot(0, 3, 0, q_unit(0, 3, 2))
        add_slot(0, 3, 1, q_unit(0, 3, 3))
        for j in range(8):
            nb, u = 2 + j // 4, j % 4
            add_slot(0, 3, 2 + j, q_unit(1, nb, u))
        for j in range(10, 16):
            add_slot(0, 3, j, transpose_dummy)
        # (1,0): transposes pair1 qg0 on j2-9; out-proj nb0 mt0-2 j10-15
        for qt in range(NSB):
            add_slot(1, 0, 4 + qt, (lambda qt=qt:
                     transpose_at(1, 0, qt, attn_tiles[(0, 1)][qt])))
        for j in range(6):
            mt, p = j // 2, j % 2
            add_slot(1, 0, 10 + j, (lambda mt=mt, p=p:
                                    out_proj_half(0, mt, p)))
        # (1,1): out-proj nb0 mt3-7, nb1 mt0-2
        for j in range(10):
            mt, p = 3 + j // 2, j % 2
            add_slot(1, 1, j, (lambda mt=mt, p=p:
                               out_proj_half(0, mt, p)))
        for j in range(6):
            mt, p = j // 2, j % 2
            add_slot(1, 1, 10 + j, (lambda mt=mt, p=p:
                                    out_proj_half(1, mt, p)))
        # (1,2): out-proj nb1 mt3; transposes pair0 qg1 j2-9; nb1 mt4-7
        add_slot(1, 2, 0, lambda: out_proj_half(1, 3, 0))
        add_slot(1, 2, 1, lambda: out_proj_half(1, 3, 1))
        for qt in range(NSB):
            add_slot(1, 2, 4 + qt, (lambda qt=qt:
                     transpose_at(0, 1, qt, attn_tiles[(1, 0)][qt])))
        for j in range(6):
            mt, p = 4 + j // 2, j % 2
            add_slot(1, 2, 10 + j, (lambda mt=mt, p=p:
                                    out_proj_half(1, mt, p)))
        # (1,3): out-proj nb1 mt7; qg1 pair0 out-proj partials
        add_slot(1, 3, 0, lambda: out_proj_half(1, 7, 0))
        add_slot(1, 3, 1, lambda: out_proj_half(1, 7, 1))
        for j in range(14):
            nb, mt = 2 + j // 8, j % 8
            add_slot(1, 3, 2 + j, (lambda nb=nb, mt=mt:
                                   out_proj_p0_store(nb, mt)))
        add_slot(1, 3, 15, lambda: out_proj_p0_store(3, 6))
        add_slot(1, 3, 15, lambda: out_proj_p0_store(3, 7))

        # ---- attention ----
        attention_all()

        # ---- tail: transposes of (qg1, pair1) + pair1 out-proj + add ----
        for qt in range(4):
            transpose_at(1, 1, qt, attn_tiles[(1, 1)][qt])
        for mt in range(8):
            out_proj_p1_add(2, mt)
            if mt < 4:
                transpose_at(1, 1, 4 + mt, attn_tiles[(1, 1)][4 + mt])
        for mt in range(8):
            out_proj_p1_add(3, mt)

    nc.compile()
    return nc


def make_in_maps(x, Wq, bq, Wk, bk, Wv, bv, Wo):
    """Shard full inputs into 8 per-core input maps."""
    import ml_dtypes
    BF = ml_dtypes.bfloat16
    scale = np.float32(1.0 / np.sqrt(DH))
    xT = [np.ascontiguousarray(x[b].T).astype(BF) for b in range(2)]
    ident = np.eye(128, dtype=np.float32).astype(BF)
    in_maps = []
    for c in range(8):
        b, g = c // 4, c % 4
        sl = slice(C * g, C * (g + 1))
        brow_g = np.concatenate([bq[sl] * scale, bk[sl]])[None, :]
        in_maps.append({
            "xT": xT[b],
            "wq": (np.ascontiguousarray(Wq[:, sl]) * scale).astype(BF),
            "wk": np.ascontiguousarray(Wk[:, sl]).astype(BF),
            "wv": np.ascontiguousarray(Wv[:, sl]).astype(BF),
            "wo": np.ascontiguousarray(Wo[sl, :]).astype(BF),
            "brow": np.ascontiguousarray(brow_g).astype(BF),
            "ident": ident,
        })
    return in_maps


def kernel(x, Wq, bq, Wk, bk, Wv, bv, Wo, bo):
    if os.environ.get("JAX_PLATFORMS") and \
            "axon" not in os.environ["JAX_PLATFORMS"]:
        os.environ.pop("JAX_PLATFORMS")
    trace = bool(os.environ.get("KERNEL_TRACE"))
    if trace:
        _install_ntff_shim()
    from concourse import bass_utils

    x = np.asarray(x, dtype=np.float32)
    in_maps = make_in_maps(
        x, np.asarray(Wq), np.asarray(bq), np.asarray(Wk), np.asarray(bk),
        np.asarray(Wv), np.asarray(bv), np.asarray(Wo))

    if "nc" not in _CACHE:
        _CACHE["nc"] = build_nc()
    res = bass_utils.run_bass_kernel_spmd(
        _CACHE["nc"], in_maps, core_ids=list(range(8)), trace=trace)
    _CACHE["exec_time_ns"] = res.exec_time_ns

    # softmax rows sum to 1, so the v-bias contributes exactly bv @ Wo
    bo_eff = (np.asarray(bo, dtype=np.float32)
              + np.asarray(bv, dtype=np.float32)
              @ np.asarray(Wo, dtype=np.float32))
    out = np.empty((2, S, D), dtype=np.float32)
    for b in range(2):
        acc = res.results[4 * b]["yT"].astype(np.float32)
        for g in range(1, 4):
            acc += res.results[4 * b + g]["yT"].astype(np.float32)
        out[b] = acc.T + bo_eff
    return out


# revision 27
# speedup vs baseline: 1.1985x; 1.0245x over previous
"""Multi-head attention (B=2, S=2048, D=1024, H=16) on 8 TRN2 NeuronCores, v3.

Sharding: data-parallel over batch (2) x tensor-parallel over head groups
(4 groups of 4 heads).  Core c = (b = c // 4, g = c % 4).

v3 design (HAM-warm dense-PE schedule):
  - All q/k/v projections in bf16; qT/kT stored PER HEAD with the 64 dh rows
    duplicated to partitions 64-127 (dup via SBUF->SBUF DMA) so the two
    512-wide q-blocks of a scores tile run CONCURRENTLY in different PE
    row-groups (tile_position auto-derived from base partitions).
  - Projections are emitted as ~1024-cycle quarter-units and spread through
    the attention j-slots so the PE never idles -> HAM stays at K=8/8
    (2.4 GHz).  Empty late slots get dummy transposes to hold the clock.
  - Act engine runs exp back-to-back ([128,1024] per (head, kt)); it is the
    steady-state bottleneck (~1.1us/instr).
  - PV in [q, dh] orientation: lhsT = pt chunk [k,128q], rhs = v_ext [k,65]
    (64 v cols + ones col -> denominators land in pv col 64).
  - normalize = DVE reciprocal + per-partition tensor_scalar_mul; PE
    transpose puts normalized attn into aT [d, q] (+v bias folded in).
  - yT written as bf16 (halves output DMA); host accumulates in fp32.
"""

import os
import sys
import types
from contextlib import ExitStack

import numpy as np

D = 1024
S = 2048
C = 256          # head cols per core (4 heads x 64)
DH = 64
NH = 4           # heads per core
QG = 1024        # q-group width
NQG = S // QG    # 2
NST = S // 128   # 16 seq tiles
NSB = QG // 128  # 8 q-subtiles per group

_CACHE = {}


def _install_ntff_shim():
    try:
        import antenv.axon_hooks  # noqa: F401
        return
    except ImportError:
        pass
    try:
        from trn_agent_boot.trn_boot import _ntff_profile_via_ctypes
        hook = _ntff_profile_via_ctypes('/opt/axon/libaxon_pjrt.so')
    except Exception:
        hook = None
    mod = types.ModuleType('antenv.axon_hooks')
    mod.get_axon_ntff_profile_hook = lambda: hook
    mod.set_axon_ntff_profile_hook = lambda h: None
    sys.modules['antenv.axon_hooks'] = mod


def build_nc():
    import concourse.bacc as bacc
    import concourse.mybir as mybir
    import concourse.tile as tile
    from concourse.bass import ts, ds

    F32 = mybir.dt.float32
    F32R = mybir.dt.float32r
    BF16 = mybir.dt.bfloat16
    ACT = mybir.ActivationFunctionType

    nc = bacc.Bacc("TRN2", target_bir_lowering=False, debug=False)
    xT = nc.dram_tensor("xT", [D, S], BF16, kind="ExternalInput")
    wq = nc.dram_tensor("wq", [D, C], BF16, kind="ExternalInput")
    wk = nc.dram_tensor("wk", [D, C], BF16, kind="ExternalInput")
    wv = nc.dram_tensor("wv", [D, C], BF16, kind="ExternalInput")
    wo = nc.dram_tensor("wo", [C, D], BF16, kind="ExternalInput")
    brow = nc.dram_tensor("brow", [1, 512], BF16, kind="ExternalInput")
    ident = nc.dram_tensor("ident", [128, 128], BF16, kind="ExternalInput")
    yT = nc.dram_tensor("yT", [D, S], BF16, kind="ExternalOutput")

    with tile.TileContext(nc) as tc, ExitStack() as ctx:
        consts = ctx.enter_context(tc.tile_pool(name="consts", bufs=1))
        sbw = ctx.enter_context(tc.tile_pool(name="weights", bufs=1))
        sbx = ctx.enter_context(tc.tile_pool(name="xsb", bufs=1))
        sbqkv = ctx.enter_context(tc.tile_pool(name="qkv", bufs=1))
        sbpt = ctx.enter_context(tc.tile_pool(name="ptp", bufs=3))
        sbat = ctx.enter_context(tc.tile_pool(name="atn", bufs=2))
        sbnrm = ctx.enter_context(tc.tile_pool(name="nrm", bufs=2))
        sby = ctx.enter_context(tc.tile_pool(name="ysb", bufs=4))
        sbtmp = ctx.enter_context(tc.tile_pool(name="tmpsb", bufs=2))
        sby0 = ctx.enter_context(tc.tile_pool(name="y0sb", bufs=1))
        # PSUM: sc 2x[128,1024] = 4 banks, pv 1x(2x[128,260]) = 2 banks,
        #       tr 1x[128,128] = 1 bank, yp 1x[128,512] = 1 bank -> 8 total
        scp = ctx.enter_context(tc.tile_pool(name="psc", bufs=2, space="PSUM"))
        pvp = ctx.enter_context(tc.tile_pool(name="ppv", bufs=1, space="PSUM"))
        trp = ctx.enter_context(tc.tile_pool(name="ptr", bufs=1, space="PSUM"))
        ypp = ctx.enter_context(tc.tile_pool(name="pyp", bufs=1, space="PSUM"))

        # ---- constants ----
        brow_sb = consts.tile([1, 512], BF16, tag="brow", name="brow_sb")
        nc.sync.dma_start(brow_sb[:], brow[:, :])
        ones_sb = consts.tile([1, 512], BF16, tag="ones", name="ones_sb")
        nc.gpsimd.memset(ones_sb[:], 1.0)
        id_sb = consts.tile([128, 128], BF16, tag="ident", name="id_sb")
        nc.sync.dma_start(id_sb[:], ident[:, :])

        # ---- input DMAs (ordered: wk, wv, x nb0, x nb1, wq, x nb2/3, wo) --
        wk_sb = [sbw.tile([128, C], BF16, tag=f"wk{i}", name=f"wk{i}")
                 for i in range(8)]
        wv_sb = [sbw.tile([128, C], BF16, tag=f"wv{i}", name=f"wv{i}")
                 for i in range(8)]
        wq_sb = [sbw.tile([128, C], BF16, tag=f"wq{i}", name=f"wq{i}")
                 for i in range(8)]
        xt_sb = [sbx.tile([128, S], BF16, tag=f"xt{i}", name=f"xt{i}")
                 for i in range(8)]
        for i in range(8):
            nc.sync.dma_start(wk_sb[i][:], wk[ts(i, 128), :])
        for i in range(8):
            nc.sync.dma_start(xt_sb[i][:, ts(0, 512)],
                              xT[ts(i, 128), ts(0, 512)])
        for i in range(8):
            nc.sync.dma_start(wq_sb[i][:], wq[ts(i, 128), :])
        for i in range(8):
            nc.sync.dma_start(xt_sb[i][:, ts(1, 512)],
                              xT[ts(i, 128), ts(1, 512)])
        for i in range(8):
            nc.sync.dma_start(wv_sb[i][:], wv[ts(i, 128), :])
        for nb in range(2, 4):
            for i in range(8):
                nc.sync.dma_start(xt_sb[i][:, ts(nb, 512)],
                                  xT[ts(i, 128), ts(nb, 512)])
        wo_sb = []
        for i in range(2):
            t = sbw.tile([128, D], BF16, tag=f"wo{i}", name=f"wo{i}")
            nc.sync.dma_start(t[:], wo[ts(i, 128), :])
            wo_sb.append(t)

        # ---- persistent activations ----
        # per-head q/k, dh rows duplicated into partitions 64-127
        qTd_sb = [sbqkv.tile([128, S], BF16, tag=f"qTd{h}", name=f"qTd{h}")
                  for h in range(NH)]
        kTd_sb = [sbqkv.tile([128, S], BF16, tag=f"kTd{h}", name=f"kTd{h}")
                  for h in range(NH)]
        v_sb = [sbqkv.tile([128, NH * 65], BF16, tag=f"v{i}", name=f"v{i}")
                for i in range(NST)]
        aT_sb = [sbqkv.tile([128, S], BF16, tag=f"aT{i}", name=f"aT{i}")
                 for i in range(2)]

        # ones columns of v_ext (col 65h+64 = 1.0)
        for st in range(NST):
            v3 = v_sb[st][:].rearrange("p (h e) -> p h e", e=65)
            nc.gpsimd.memset(v3[:, :, 64:65], 1.0)

        # ---- projection quarter-units (~1024 PE cycles each) ----
        open_pj = {}

        def proj_unit(which, mt, nb, u, pool=None, fast_dup=False):
            """2 of the 8 k-tile matmuls of one [128,512] q/k proj stripe;
            u==3 adds the bias via a K=1 ones-row matmul, casts to bf16
            once on DVE, then sprays the per-head row-dup copies via DMA."""
            w_sb, dsts, bcol = {
                "q": (wq_sb, qTd_sb, 0), "k": (wk_sb, kTd_sb, 256)}[which]
            key = (which, mt, nb)
            if u == 0:
                p = pool if pool is not None else ypp
                open_pj[key] = p.tile([128, 512], F32,
                                      tag="sc" if p is scp else "yp",
                                      name=f"pj_{which}{mt}{nb}")
            pj = open_pj[key]
            for kt in range(2 * u, 2 * u + 2):
                nc.tensor.matmul(
                    pj[:],
                    lhsT=w_sb[kt][:, ts(mt, 128)],
                    rhs=xt_sb[kt][:, ts(nb, 512)],
                    start=(kt == 0), stop=False,
                )
            if u == 3:
                nc.tensor.matmul(
                    pj[:],
                    lhsT=brow_sb[0:1, ds(bcol + 128 * mt, 128)],
                    rhs=ones_sb[0:1, :],
                    start=False, stop=True,
                )
                tmp = sbtmp.tile([128, 512], BF16, tag="pt16",
                                 name=f"pt16_{which}{mt}{nb}")
                nc.vector.tensor_copy(tmp[:], pj[:])
                eng = nc.scalar if fast_dup else nc.sync
                for hh in range(2):
                    dst = dsts[2 * mt + hh]
                    for rep in range(2):
                        eng.dma_start(
                            dst[64 * rep:64 * rep + 64, ts(nb, 512)],
                            tmp[64 * hh:64 * hh + 64, :])
                del open_pj[key]

        open_vp = {}

        def proj_v_half(st, half, pool=None):
            """half a seq-tile of v projection (4 of 8 k-tiles)."""
            if half == 0:
                p = pool if pool is not None else ypp
                open_vp[st] = p.tile([128, C], F32,
                                     tag="sc" if p is scp else "yp",
                                     name=f"vp{st}")
            vp = open_vp[st]
            for kt in range(4 * half, 4 * half + 4):
                nc.tensor.matmul(
                    vp[:],
                    lhsT=xt_sb[kt][:, ts(st, 128)],
                    rhs=wv_sb[kt][:],
                    start=(kt == 0), stop=(kt == 7),
                )
            if half == 1:
                v3 = v_sb[st][:].rearrange("p (h e) -> p h e", e=65)
                nc.vector.tensor_copy(
                    v3[:, :, 0:64],
                    vp[:].rearrange("p (h e) -> p h e", e=64))
                del open_vp[st]

        def transpose_at(pair, qg, qt, at_tile):
            """attn [128q,128d] -> aT[pair][...] via PE transpose + copy.
            v-bias is folded into the host-side output bias (softmax rows
            sum to 1, so attn@(v+bv) = attn@v + bv)."""
            tr = trp.tile([128, 128], BF16, tag="tr", name="tr")
            nc.tensor.transpose(tr[:], at_tile[:], id_sb[:])
            nc.vector.tensor_copy(
                aT_sb[pair][:, ds(qg * QG + qt * 128, 128)], tr[:])

        def transpose_dummy():
            """PE filler to keep the HAM clock-gate open in empty slots."""
            tr = trp.tile([128, 128], BF16, tag="tr", name="trd")
            nc.tensor.transpose(tr[:], id_sb[:], id_sb[:])

        # qg1 out-proj is split: pair0 partial computed early (hidden in
        # (1,3) slots, stashed in SBUF), pair1 matmul + add + store at tail.
        y0_sb = {}

        def out_proj_p0_store(nb, mt):
            yp = ypp.tile([128, 512], F32, tag="yp", name=f"y0p{nb}{mt}")
            nc.tensor.matmul(
                yp[:], lhsT=wo_sb[0][:, ts(mt, 128)],
                rhs=aT_sb[0][:, ts(nb, 512)], start=True, stop=True)
            t = sby0.tile([128, 512], F32, tag=f"y0_{nb}_{mt}",
                          name=f"y0_{nb}_{mt}")
            nc.vector.tensor_copy(t[:], yp[:])
            y0_sb[(nb, mt)] = t

        def out_proj_p1_add(nb, mt):
            yp = scp.tile([128, 512], F32, tag="sc", name=f"y1p{nb}{mt}")
            nc.tensor.matmul(
                yp[:], lhsT=wo_sb[1][:, ts(mt, 128)],
                rhs=aT_sb[1][:, ts(nb, 512)], start=True, stop=True)
            yt = sby.tile([128, 512], BF16, tag="yt", name="yt")
            nc.vector.tensor_add(yt[:], yp[:], y0_sb[(nb, mt)][:])
            nc.scalar.dma_start(yT[ts(mt, 128), ts(nb, 512)], yt[:])

        open_yp = {}

        def out_proj_half(nb, mt, p, tail=False):
            """one of the two accumulation matmuls of an out-proj stripe."""
            key = (nb, mt)
            if p == 0:
                pool = scp if tail else ypp
                open_yp[key] = pool.tile([128, 512], F32,
                                         tag="sc" if tail else "yp",
                                         name=f"yp{nb}{mt}")
            yp = open_yp[key]
            nc.tensor.matmul(
                yp[:],
                lhsT=wo_sb[p][:, ts(mt, 128)],
                rhs=aT_sb[p][:, ts(nb, 512)],
                start=(p == 0), stop=(p == 1),
            )
            if p == 1:
                yt = sby.tile([128, 512], BF16, tag="yt", name="yt")
                nc.vector.tensor_copy(yt[:], yp[:])
                eng = nc.scalar if tail else nc.sync
                eng.dma_start(yT[ts(mt, 128), ts(nb, 512)], yt[:])
                del open_yp[key]

        # ---- interleave schedule ----
        slots = {(qg, h): {} for qg in range(NQG) for h in range(NH)}

        def add_slot(qg, h, kt, fn):
            slots[(qg, h)].setdefault(kt, []).append(fn)

        # per-(qg,pair) attn tiles, filled by norm, consumed by transpose
        attn_tiles = {}

        def norm_pair_writes(qg, h, pva, pvb):
            pair = h // 2
            if (qg, pair) not in attn_tiles:
                attn_tiles[(qg, pair)] = [
                    sbat.tile([128, 128], BF16, tag=f"at{qt}", name=f"at{qt}")
                    for qt in range(NSB)]
            tiles = attn_tiles[(qg, pair)]
            col = 64 * (h % 2)
            pa3 = pva[:].rearrange("p (s e) -> p s e", e=65)
            pb3 = pvb[:].rearrange("p (s e) -> p s e", e=65)
            recip = sbnrm.tile([128, 8], F32, tag="rc", name="rc")
            nc.vector.reciprocal(recip[:, 0:4], pa3[:, :, 64])
            nc.vector.reciprocal(recip[:, 4:8], pb3[:, :, 64])
            for qs in range(NSB):
                src3 = pa3 if qs < 4 else pb3
                nc.vector.tensor_scalar_mul(
                    tiles[qs][:, col:col + 64],
                    src3[:, qs % 4, 0:64],
                    recip[:, qs:qs + 1])

        def attention_all():
            """single software pipeline over all (qg, h, kt): iteration t
            emits exp(t-1) FIRST (the act engine's coalesced PE-semaphore
            threshold then only covers work finished a full period ago),
            then scores(t), slot fillers, pv(t-2).  Flattening across head
            boundaries removes per-head pipeline drain/refill bubbles."""
            T = NQG * NH * NST
            state, pts, scs = {}, {}, {}

            def hq(t):
                head = t // NST
                return head // NH, head % NH, t % NST

            for t in range(T + 2):
                if 1 <= t <= T:
                    pt = sbpt.tile([128, QG], BF16, tag="pt", name="pt")
                    pts[t - 1] = pt
                    nc.scalar.activation(pt[:], scs.pop(t - 1)[:], ACT.Exp)
                if t < T:
                    qg, h, j = hq(t)
                    ktd, qtd = kTd_sb[h], qTd_sb[h]
                    sc = scp.tile([128, QG], F32, tag="sc", name="sc_at")
                    scs[t] = sc
                    # two q-blocks in different PE row groups -> concurrent
                    for qb in range(2):
                        rg = 64 * qb
                        nc.tensor.matmul(
                            sc[:, ts(qb, 512)],
                            lhsT=ktd[rg:rg + 64, ts(j, 128)],
                            rhs=qtd[rg:rg + 64,
                                    ds(qg * QG + qb * 512, 512)],
                            start=True, stop=True,
                        )
                    for fn in slots[(qg, h)].get(j, ()):
                        fn()
                t2 = t - 2
                if t2 >= 0:
                    qg2, h2, j2 = hq(t2)
                    if j2 == 0:
                        state[(qg2, h2)] = (
                            pvp.tile([128, 4 * 65], F32, tag="pva",
                                     name="pva"),
                            pvp.tile([128, 4 * 65], F32, tag="pvb",
                                     name="pvb"))
                    pva, pvb = state[(qg2, h2)]
                    ptt = pts.pop(t2)
                    for qs in range(NSB):
                        pvt = pva if qs < 4 else pvb
                        nc.tensor.matmul(
                            pvt[:, ds((qs % 4) * 65, 65)],
                            lhsT=ptt[:, ts(qs, 128)],
                            rhs=v_sb[j2][:, ds(65 * h2, 65)],
                            start=(j2 == 0 and qs % 4 == 0),
                            stop=(j2 == NST - 1 and qs % 4 == 3),
                        )
                    if j2 == NST - 1:
                        norm_pair_writes(qg2, h2, pva, pvb)
                        del state[(qg2, h2)]

        # ---- lead-in: only what scores(0,0) j0 needs ----
        # k mt0 nb0 (kT head0 cols 0-512) + q mt0 nb0+nb1 (qT head0, qg0).
        for u in range(4):
            proj_unit("k", 0, 0, u, pool=scp, fast_dup=True)
        for u in range(4):
            proj_unit("q", 0, 0, u, pool=ypp, fast_dup=True)
        for u in range(4):
            proj_unit("q", 0, 1, u, pool=scp, fast_dup=True)

        # ---- slot fillers ----
        def k_unit(mt, nb, u):
            return lambda: proj_unit("k", mt, nb, u)

        def q_unit(mt, nb, u):
            return lambda: proj_unit("q", mt, nb, u)

        def v_full(st):
            return [lambda st=st: proj_v_half(st, 0),
                    lambda st=st: proj_v_half(st, 1)]

        # (0,0): k mt0 catch-up just in time (nb1 by j4, nb2 by j8, nb3 by
        # j12) + v st0-15 just in time (st_i by pv at j=i+2).
        plan00 = {0: [k_unit(0, 1, u) for u in range(4)],
                  1: v_full(0), 2: v_full(1),
                  11: v_full(10) + v_full(15),
                  12: v_full(11), 13: v_full(12), 14: v_full(13),
                  15: v_full(14)}
        for j in range(4):
            plan00[3 + j] = v_full(2 + j) + [k_unit(0, 2, j)]
            plan00[7 + j] = v_full(6 + j) + [k_unit(0, 3, j)]
        for j, fns in plan00.items():
            for fn in fns:
                add_slot(0, 0, j, fn)
        # (0,1): k mt1 nb0 + q mt1 nb0/nb1; light second half
        for u in range(4):
            add_slot(0, 1, u, k_unit(1, 0, u))
            add_slot(0, 1, 4 + u, q_unit(1, 0, u))
            add_slot(0, 1, 8 + u, q_unit(1, 1, u))
        for j in range(12, 16):
            add_slot(0, 1, j, transpose_dummy)
        # (0,2): k mt1 nb1/2/3 just in time + transposes pair0 qg0
        for u in range(4):
            add_slot(0, 2, u // 2, k_unit(1, 1, u))
            add_slot(0, 2, 4 + u // 2, k_unit(1, 2, u))
            add_slot(0, 2, 8 + u // 2, k_unit(1, 3, u))
        for qt in range(NSB):
            j = (2, 3, 6, 7, 10, 11, 12, 13)[qt]
            add_slot(0, 2, j, (lambda qt=qt:
                     transpose_at(0, 0, qt, attn_tiles[(0, 0)][qt])))
        for j in (14, 15):
            add_slot(0, 2, j, transpose_dummy)
        # (0,3): q mt0 nb2/nb3 (for (1,0)), q mt1 nb2/nb3 (for (1,2))
        for u in range(4):
            add_slot(0, 3, u, q_unit(0, 2, u))
            add_slot(0, 3, 4 + u, q_unit(0, 3, u))
            add_slot(0, 3, 8 + u, q_unit(1, 2, u))
            add_slot(0, 3, 12 + u, q_unit(1, 3, u))
        # out-proj halves of qg0, spread over (1,0) j11 .. (1,3) j2
        op = [(nb, mt, p) for nb in (0, 1) for mt in range(8)
              for p in (0, 1)]

        def op_slot(qg, h, j, idx):
            nb, mt, p = op[idx]
            add_slot(qg, h, j, (lambda nb=nb, mt=mt, p=p:
                                out_proj_half(nb, mt, p)))

        # (1,0): transposes pair1 qg0 j3-10; out-proj halves 0-4 j11-15
        for qt in range(NSB):
            add_slot(1, 0, 3 + qt, (lambda qt=qt:
                     transpose_at(1, 0, qt, attn_tiles[(0, 1)][qt])))
        for j in range(5):
            op_slot(1, 0, 11 + j, j)
        # (1,1): out-proj halves 5-20
        for j in range(16):
            op_slot(1, 1, j, 5 + j)
        # (1,2): out-proj 21-23; transposes pair0 qg1 j3-10; 24-28
        for j in range(3):
            op_slot(1, 2, j, 21 + j)
        for qt in range(NSB):
            add_slot(1, 2, 3 + qt, (lambda qt=qt:
                     transpose_at(0, 1, qt, attn_tiles[(1, 0)][qt])))
        for j in range(5):
            op_slot(1, 2, 11 + j, 24 + j)
        # (1,3): out-proj 29-31; qg1 pair0 partials; dummies
        for j in range(3):
            op_slot(1, 3, j, 29 + j)
        for j in range(16):
            nb, mt = 2 + j // 8, j % 8
            add_slot(1, 3, 3 + j // 2, (lambda nb=nb, mt=mt:
                                        out_proj_p0_store(nb, mt)))
        for j in range(11, 16):
            add_slot(1, 3, j, transpose_dummy)

        # ---- attention ----
        attention_all()

        # ---- tail: transposes of (qg1, pair1) + pair1 out-proj + add ----
        for qt in range(4):
            transpose_at(1, 1, qt, attn_tiles[(1, 1)][qt])
        for mt in range(8):
            out_proj_p1_add(2, mt)
            if mt < 4:
                transpose_at(1, 1, 4 + mt, attn_tiles[(1, 1)][4 + mt])
        for mt in range(8):
            out_proj_p1_add(3, mt)

    nc.compile()
    return nc


def make_in_maps(x, Wq, bq, Wk, bk, Wv, bv, Wo):
    """Shard full inputs into 8 per-core input maps."""
    import ml_dtypes
    BF = ml_dtypes.bfloat16
    scale = np.float32(1.0 / np.sqrt(DH))
    xT = [np.ascontiguousarray(x[b].T).astype(BF) for b in range(2)]
    ident = np.eye(128, dtype=np.float32).astype(BF)
    in_maps = []
    for c in range(8):
        b, g = c // 4, c % 4
        sl = slice(C * g, C * (g + 1))
        brow_g = np.concatenate([bq[sl] * scale, bk[sl]])[None, :]
        in_maps.append({
            "xT": xT[b],
            "wq": (np.ascontiguousarray(Wq[:, sl]) * scale).astype(BF),
            "wk": np.ascontiguousarray(Wk[:, sl]).astype(BF),
            "wv": np.ascontiguousarray(Wv[:, sl]).astype(BF),
            "wo": np.ascontiguousarray(Wo[sl, :]).astype(BF),
            "brow": np.ascontiguousarray(brow_g).astype(BF),
            "ident": ident,
        })
    return in_maps


def kernel(x, Wq, bq, Wk, bk, Wv, bv, Wo, bo):
    if os.environ.get("JAX_PLATFORMS") and \
            "axon" not in os.environ["JAX_PLATFORMS"]:
        os.environ.pop("JAX_PLATFORMS")
    trace = bool(os.environ.get("KERNEL_TRACE"))
    if trace:
        _install_ntff_shim()
    from concourse import bass_utils

    x = np.asarray(x, dtype=np.float32)
    in_maps = make_in_maps(
        x, np.asarray(Wq), np.asarray(bq), np.asarray(Wk), np.asarray(bk),
        np.asarray(Wv), np.asarray(bv), np.asarray(Wo))

    if "nc" not in _CACHE:
        _CACHE["nc"] = build_nc()
    res = bass_utils.run_bass_kernel_spmd(
        _CACHE["nc"], in_maps, core_ids=list(range(8)), trace=trace)
    _CACHE["exec_time_ns"] = res.exec_time_ns

    # softmax rows sum to 1, so the v-bias contributes exactly bv @ Wo
    bo_eff = (np.asarray(bo, dtype=np.float32)
              + np.asarray(bv, dtype=np.float32)
              @ np.asarray(Wo, dtype=np.float32))
    out = np.empty((2, S, D), dtype=np.float32)
    for b in range(2):
        acc = res.results[4 * b]["yT"].astype(np.float32)
        for g in range(1, 4):
            acc += res.results[4 * b + g]["yT"].astype(np.float32)
        out[b] = acc.T + bo_eff
    return out
